# revision 27
# baseline (speedup 1.0000x reference)
"""Trainium2 Bass kernel for nn_GAT_T (2x GATConv + dense self-attention hybrid).

Sharding across 8 NeuronCores: core c owns nodes [1024c, 1024(c+1)).
 - GATConv: dst-node sharded, dense masked-softmax formulation. Per-layer
   h (all nodes) comes from an own-slice matmul + AllGather; the adjacency
   mask is built just-in-time inside the GAT loop by gpsimd local_scatter
   (no DRAM round trip). The LeakyReLU+exp is factored exactly as
   exp(lrelu(ss+sd)) = max(e^ss * e^sd, e^{.2ss} * e^{.2sd}) so the inner
   loop is 3 fused DVE ops (no ScalarE).
 - Dense NxN attention: query-row sharded; K/V own-slice + AllGather during
   GAT1, streamed in 512KB tiles during the attention phase.
 - Collectives (order): ssrc1 (4KB), h1 (1MB/core), k, v during prologue/
   GAT1; ssrc2 (4KB), h2 (1MB/core) during attention.
Heavy matmuls in bf16 with fp32 PSUM accumulation. Softmax computed without
max-subtraction (logits are O(+-6); mathematically identical).
"""

import numpy as np

NA, NB = 4096, 4096
N = NA + NB
IN, H = 256, 512
N_CORES = 8
NO = N // N_CORES      # 1024 nodes per core
KT = N // 128          # 64 src chunks
KP = KT // 2           # 32 src chunk-pairs
NEG_SLOPE = 0.2

TRACE = False
LAST_EXEC_NS = None
_LAST_RES = None
_CACHE = {}


def _install_trace_hook():
    import sys, types
    if "antenv.axon_hooks" in sys.modules:
        return
    try:
        mod = types.ModuleType("antenv.axon_hooks")
        mod._hook = None
        mod.set_axon_ntff_profile_hook = lambda h: setattr(mod, "_hook", h)
        mod.get_axon_ntff_profile_hook = lambda: mod._hook
        sys.modules["antenv.axon_hooks"] = mod
        from trn_agent_boot.trn_boot import _ntff_profile_via_ctypes
        mod.set_axon_ntff_profile_hook(
            _ntff_profile_via_ctypes("/opt/axon/libaxon_pjrt.so"))
    except Exception:
        pass


def _build(W0, W1):
    import concourse.bacc as bacc
    import concourse.mybir as mybir
    import concourse.tile as tile

    f32 = mybir.dt.float32
    bf16 = mybir.dt.bfloat16
    i16 = mybir.dt.int16
    AF = mybir.ActivationFunctionType
    ALU = mybir.AluOpType

    nc = bacc.Bacc("TRN2", target_bir_lowering=False, debug=False,
                   num_devices=N_CORES)

    def inp(name, shape, dt=f32):
        return nc.dram_tensor(name, shape, dt, kind="ExternalInput")

    # weights arrive pre-cast to bf16 from the host so every load is a plain
    # HWDGE DMA (keeps the gpsimd queue free for the barrier + cc triggers)
    xoT = inp("xoT", [IN, NO], bf16)
    win_o = inp("win_o", [IN, H], bf16); win2_o = inp("win2_o", [IN, H], bf16)
    bin_o = inp("bin_o", [H, 1]); bin2_o = inp("bin2_o", [H, 1])
    Wg1 = inp("Wg1", [H, H], bf16); Wg1_T = inp("Wg1_T", [H, H], bf16)
    A1 = inp("A1", [H, 2], bf16)
    Wg2 = inp("Wg2", [H, H], bf16); Wg2_T = inp("Wg2_T", [H, H], bf16)
    A2 = inp("A2", [H, 2], bf16)
    bg1 = inp("bg1", [H, 1]); bg2 = inp("bg2", [H, 1])
    WqT = inp("WqT", [H, H], bf16); WkT = inp("WkT", [H, H], bf16)
    WvT = inp("WvT", [H, H], bf16)
    bq = inp("bq", [H, 1]); bk = inp("bk", [H, 1]); bv = inp("bv", [H, 1])
    WoT = inp("WoT", [H, H], bf16); bo = inp("bo", [H, 1])
    sc_idx0 = inp("sc_idx0", [128, KP * W0], i16)
    sc_cnt0 = inp("sc_cnt0", [128, KP * W0], bf16)
    sc_idx1 = inp("sc_idx1", [128, KP * W1], i16)
    sc_cnt1 = inp("sc_cnt1", [128, KP * W1], bf16)

    out_l = nc.dram_tensor("out_l", [4, 128, NO], f32, kind="ExternalOutput")
    out_g = nc.dram_tensor("out_g", [4, 128, NO], f32, kind="ExternalOutput")

    RG = [list(range(N_CORES))]
    SCL = 1.0 / float(np.sqrt(H))

    with tile.TileContext(nc) as tc:
        with (
            tc.tile_pool(name="wp", bufs=1) as wp,
            tc.tile_pool(name="apool", bufs=1) as ap,
            tc.tile_pool(name="own", bufs=1) as op_,
            tc.tile_pool(name="hp", bufs=1) as hp,
            tc.tile_pool(name="sp", bufs=3) as sp,
            tc.tile_pool(name="rp", bufs=2) as rp,
            tc.tile_pool(name="lp", bufs=3) as lp,
            tc.tile_pool(name="pp", bufs=1, space="PSUM") as pp,
            tc.tile_pool(name="ppmm", bufs=2, space="PSUM") as ppmm,
            tc.tile_pool(name="dram", bufs=1, space="DRAM") as dp,
        ):
            # ---------- load weights (HWDGE; already bf16) ----------
            def w16(dram, rows, cols, tag):
                t = wp.tile([128, rows // 128, cols], bf16, tag=tag)
                nc.sync.dma_start(
                    t[:], dram[:].rearrange("(a p) c -> p a c", p=128))
                return t

            def bias32(dram, tag):
                t = wp.tile([128, H // 128], f32, tag=tag)
                nc.scalar.dma_start(
                    t[:], dram[:].rearrange("(a p) one -> p (a one)", p=128))
                return t

            # own x first (scalar queue), prologue-critical weights (sync)
            xo = lp.tile([128, 2, NO], bf16, tag="xo", bufs=1)
            nc.scalar.dma_start(
                xo[:], xoT[:].rearrange("(a p) c -> p a c", p=128))
            wo_ = w16(win_o, IN, H, "wo_")
            bof_ = bias32(bin_o, "bof_")
            g1 = w16(Wg1, H, H, "g1")
            a1 = w16(A1, H, 2, "a1")
            g1T = w16(Wg1_T, H, H, "g1T")
            bg1f = bias32(bg1, "bg1f")
            w2o_ = w16(win2_o, IN, H, "w2o_")
            b2of_ = bias32(bin2_o, "b2of_")
            qT = w16(WqT, H, H, "qT"); bqf = bias32(bq, "bqf")
            kT = w16(WkT, H, H, "kT"); bkf = bias32(bk, "bkf")
            vT = w16(WvT, H, H, "vT")

            ones_r = wp.tile([1, 128], f32, tag="ones_r")
            nc.vector.memset(ones_r[:], 1.0)
            ones_cb = wp.tile([128, 1], bf16, tag="ones_cb")
            nc.vector.memset(ones_cb[:], 1.0)

            sci = [wp.tile([128, KP * W0], i16, tag="sci0", name="sci0"),
                   wp.tile([128, KP * W1], i16, tag="sci1", name="sci1")]
            scc = [wp.tile([128, KP * W0], bf16, tag="scc0", name="scc0"),
                   wp.tile([128, KP * W1], bf16, tag="scc1", name="scc1")]
            nc.scalar.dma_start(sci[0][:], sc_idx0[:])
            nc.scalar.dma_start(scc[0][:], sc_cnt0[:])
            nc.scalar.dma_start(sci[1][:], sc_idx1[:])
            nc.scalar.dma_start(scc[1][:], sc_cnt1[:])

            # bv as a broadcast row [128, 512] f32
            bvrow = wp.tile([1, H], f32, tag="bvrow")
            nc.scalar.dma_start(bvrow[:], bv[:].rearrange("f o -> o f"))

            # ---------- internal DRAM ----------
            h1o_b = dp.tile([8, 128, 512], bf16, tag="h1ob")
            h1g = dp.tile([N_CORES, 8, 128, 512], bf16, tag="h1g",
                          addr_space="Shared")
            s1o = dp.tile([1, NO], f32, tag="s1o")
            s1g = dp.tile([N_CORES, 1, NO], f32, tag="s1g", addr_space="Shared")
            ko_b = dp.tile([4, 128, NO], bf16, tag="kob")
            kg = dp.tile([N_CORES, 4, 128, NO], bf16, tag="kg",
                         addr_space="Shared")
            vo_b = dp.tile([8, 128, 512], bf16, tag="vob")
            vg = dp.tile([N_CORES, 8, 128, 512], bf16, tag="vg",
                         addr_space="Shared")
            s2o = dp.tile([1, NO], f32, tag="s2o")
            s2g = dp.tile([N_CORES, 1, NO], f32, tag="s2g", addr_space="Shared")
            h2o_b = dp.tile([8, 128, 512], bf16, tag="h2ob")
            h2g = dp.tile([N_CORES, 8, 128, 512], bf16, tag="h2g",
                          addr_space="Shared")

            # ---------- prologue: own input linears ----------
            l0o = op_.tile([128, 4, NO], bf16, tag="l0o")
            g0o = op_.tile([128, 4, NO], bf16, tag="g0o")
            for n2 in range(2):
                for m in range(4):
                    ps = ppmm.tile([128, 512], f32, tag="mm")
                    for k2 in range(2):
                        nc.tensor.matmul(
                            ps[:], lhsT=wo_[:, k2, 128 * m:128 * (m + 1)],
                            rhs=xo[:, k2, 512 * n2:512 * (n2 + 1)],
                            start=(k2 == 0), stop=(k2 == 1))
                    nc.vector.tensor_scalar_add(
                        l0o[:, m, 512 * n2:512 * (n2 + 1)], ps[:], bof_[:, m:m + 1])

            def compute_wsd(gw, aw, tag):
                wsd = ap.tile([128, 4, 2], bf16, tag=tag)
                for m in range(4):
                    psw = ppmm.tile([128, 2], f32, tag="mm")
                    for k2 in range(4):
                        nc.tensor.matmul(
                            psw[:], lhsT=gw[:, k2, 128 * m:128 * (m + 1)],
                            rhs=aw[:, k2, :], start=(k2 == 0), stop=(k2 == 3))
                    nc.vector.tensor_copy(wsd[:, m, :], psw[:])
                return wsd

            def ssrc_own(wsd, lown, s_dram):
                """own-node ssrc rows -> s_dram [1, NO] f32."""
                for n2 in range(2):
                    psd = pp.tile([1, 512], f32, tag="den")
                    for k2 in range(4):
                        nc.tensor.matmul(
                            psd[:], lhsT=wsd[:, k2, 0:1],
                            rhs=lown[:, k2, 512 * n2:512 * (n2 + 1)],
                            start=(k2 == 0), stop=(k2 == 3))
                    row = rp.tile([1, 512], f32, tag="row")
                    nc.vector.tensor_copy(row[:], psd[:])
                    nc.sync.dma_start(
                        s_dram[:, 512 * n2:512 * (n2 + 1)], row[:])

            wsd1 = compute_wsd(g1, a1, "wsd1")
            ssrc_own(wsd1, l0o, s1o)
            nc.gpsimd.collective_compute(
                "AllGather", mybir.AluOpType.bypass,
                replica_groups=RG, ins=[s1o.opt()], outs=[s1g.opt()])

            # ---------- h1 own (node-major) -> AllGather ----------
            for tp in range(4):
                st2 = sp.tile([128, 2, 512], bf16, tag="stg2", bufs=2)
                for ti in range(2):
                    t = 2 * tp + ti
                    ps = ppmm.tile([128, 512], f32, tag="mm")
                    for k2 in range(4):
                        nc.tensor.matmul(
                            ps[:], lhsT=l0o[:, k2, 128 * t:128 * (t + 1)],
                            rhs=g1T[:, k2, :], start=(k2 == 0), stop=(k2 == 3))
                    nc.vector.tensor_copy(st2[:, ti, :], ps[:])
                nc.sync.dma_start(
                    h1o_b[2 * tp:2 * tp + 2, :, :].rearrange("a p c -> p a c"),
                    st2[:])
            nc.gpsimd.collective_compute(
                "AllGather", mybir.AluOpType.bypass,
                replica_groups=RG, ins=[h1o_b.opt()], outs=[h1g.opt()])

            # ---------- g0o, q, k, v own + AllGather k/v ----------
            for n2 in range(2):
                for m in range(4):
                    ps = ppmm.tile([128, 512], f32, tag="mm")
                    for k2 in range(2):
                        nc.tensor.matmul(
                            ps[:], lhsT=w2o_[:, k2, 128 * m:128 * (m + 1)],
                            rhs=xo[:, k2, 512 * n2:512 * (n2 + 1)],
                            start=(k2 == 0), stop=(k2 == 1))
                    nc.vector.tensor_scalar_add(
                        g0o[:, m, 512 * n2:512 * (n2 + 1)], ps[:], b2of_[:, m:m + 1])

            q16 = ap.tile([128, 4, NO], bf16, tag="q16")
            for n2 in range(2):
                for m in range(4):
                    ps = ppmm.tile([128, 512], f32, tag="mm")
                    for k2 in range(4):
                        nc.tensor.matmul(
                            ps[:], lhsT=qT[:, k2, 128 * m:128 * (m + 1)],
                            rhs=g0o[:, k2, 512 * n2:512 * (n2 + 1)],
                            start=(k2 == 0), stop=(k2 == 3))
                    nc.vector.tensor_scalar_add(
                        q16[:, m, 512 * n2:512 * (n2 + 1)], ps[:], bqf[:, m:m + 1])

            for n2 in range(2):
                for mp in range(2):
                    st2 = sp.tile([128, 2, 512], bf16, tag="stg2", bufs=2)
                    for mi in range(2):
                        m = 2 * mp + mi
                        ps = ppmm.tile([128, 512], f32, tag="mm")
                        for k2 in range(4):
                            nc.tensor.matmul(
                                ps[:], lhsT=kT[:, k2, 128 * m:128 * (m + 1)],
                                rhs=g0o[:, k2, 512 * n2:512 * (n2 + 1)],
                                start=(k2 == 0), stop=(k2 == 3))
                        nc.vector.tensor_scalar_add(
                            st2[:, mi, :], ps[:], bkf[:, m:m + 1])
                    nc.sync.dma_start(
                        ko_b[2 * mp:2 * mp + 2, :, 512 * n2:512 * (n2 + 1)]
                        .rearrange("a p c -> p a c"), st2[:])

            # bv broadcast [128, 512]
            pb = ppmm.tile([128, H], f32, tag="mm")
            nc.tensor.matmul(pb[:], lhsT=ones_r[:], rhs=bvrow[:], start=True,
                             stop=True)
            bvb = ap.tile([128, H], f32, tag="bvb")
            nc.vector.tensor_copy(bvb[:], pb[:])
            for tp in range(4):
                st2 = sp.tile([128, 2, 512], bf16, tag="stg2", bufs=2)
                for ti in range(2):
                    t = 2 * tp + ti
                    ps = ppmm.tile([128, 512], f32, tag="mm")
                    for k2 in range(4):
                        nc.tensor.matmul(
                            ps[:], lhsT=g0o[:, k2, 128 * t:128 * (t + 1)],
                            rhs=vT[:, k2, :], start=(k2 == 0), stop=(k2 == 3))
                    nc.vector.tensor_add(st2[:, ti, :], ps[:], bvb[:])
                nc.sync.dma_start(
                    vo_b[2 * tp:2 * tp + 2, :, :].rearrange("a p c -> p a c"),
                    st2[:])
            nc.gpsimd.collective_compute(
                "AllGather", mybir.AluOpType.bypass,
                replica_groups=RG, ins=[ko_b.opt()], outs=[kg.opt()])
            nc.gpsimd.collective_compute(
                "AllGather", mybir.AluOpType.bypass,
                replica_groups=RG, ins=[vo_b.opt()], outs=[vg.opt()])

            # layer-2 / attention-epilogue weights, reusing layer-1 slots
            # (issued after the collective triggers so the WAR waits cannot
            # delay them; the donors' last reads are all in the prologue)
            g2 = w16(Wg2, H, H, "g1"); g2T = w16(Wg2_T, H, H, "g1T")
            a2 = w16(A2, H, 2, "a1")
            bg2f = bias32(bg2, "bg2f")
            oT = w16(WoT, H, H, "kT")
            bof2 = bias32(bo, "bof2")

            # ---------- helpers ----------
            def compute_sdb(wsd, lown, tag):
                """sdst over own nodes, broadcast to [128, NO] f32."""
                sdb = ap.tile([128, NO], bf16, tag=tag)
                for n2 in range(2):
                    psd = pp.tile([1, 512], f32, tag="den")
                    for k2 in range(4):
                        nc.tensor.matmul(
                            psd[:], lhsT=wsd[:, k2, 1:2],
                            rhs=lown[:, k2, 512 * n2:512 * (n2 + 1)],
                            start=(k2 == 0), stop=(k2 == 3))
                    row = rp.tile([1, 512], f32, tag="row")
                    nc.vector.tensor_copy(row[:], psd[:])
                    psb = ppmm.tile([128, 512], f32, tag="mm")
                    nc.tensor.matmul(psb[:], lhsT=ones_r[:], rhs=row[:],
                                     start=True, stop=True)
                    nc.vector.tensor_copy(sdb[:, 512 * n2:512 * (n2 + 1)], psb[:])
                return sdb

            def exp_factors(sdb, s_g):
                """F1/F2 [128, NO] bf16 (exp of sdst, exp of .2 sdst) and
                E1/E2 [128, KT] f32 (exp of ssrc) from the gathered ssrc."""
                F1 = ap.tile([128, NO], bf16, tag="F1", name="F1")
                F2 = ap.tile([128, NO], bf16, tag="F2", name="F2")
                nc.scalar.activation(F1[:], sdb[:], AF.Exp)
                nc.scalar.activation(F2[:], sdb[:], AF.Exp, scale=NEG_SLOPE)
                sc = ap.tile([128, KT], f32, tag="sc", name="sc")
                nc.scalar.dma_start(
                    sc[:], s_g[:].rearrange("c o (a p) -> p (c a o)", p=128))
                E1 = ap.tile([128, KT], f32, tag="E1", name="E1")
                E2 = ap.tile([128, KT], f32, tag="E2", name="E2")
                nc.scalar.activation(E1[:], sc[:], AF.Exp)
                nc.scalar.activation(E2[:], sc[:], AF.Exp, scale=NEG_SLOPE)
                return F1, F2, E1, E2, sc

            def load_hres(h_g, queue, tag):
                hts = []
                for c in range(N_CORES):
                    ht = hp.tile([128, 8, 512], bf16, tag=f"{tag}{c}")
                    queue.dma_start(
                        ht[:], h_g[c, :, :, :].rearrange("a p c -> p a c"))
                    hts.append(ht)
                return hts

            def gat_loop(hts, F1, F2, E1, E2, sdb, ssc, li, write_out):
                # per chunk-pair: ki=0 attention weights on ScalarE
                # (Prelu+Exp) with the mask multiply on gpsimd; ki=1 on DVE
                # (two fused stt + max). Balances all four engines under PE.
                for j in range(2):
                    Wj = W0 if j == 0 else W1
                    aggs = [pp.tile([128, 512], f32, tag=f"agg{m}",
                                    name=f"agg{li}{m}") for m in range(4)]
                    wsum = rp.tile([128, 512], bf16, tag="wsum")
                    for kp in range(KP):
                        mf = sp.tile([128, 2, 512], bf16, tag="mask", bufs=3)
                        nc.gpsimd.local_scatter(
                            out_ap=mf[:], data_ap=scc[j][:, kp * Wj:(kp + 1) * Wj],
                            idxs_ap=sci[j][:, kp * Wj:(kp + 1) * Wj],
                            channels=128, num_elems=NO, num_idxs=Wj)
                        k0 = 2 * kp
                        et = sp.tile([128, 512], f32, tag="et", bufs=3)
                        nc.scalar.activation(
                            et[:], sdb[:, 512 * j:512 * (j + 1)], AF.Prelu,
                            bias=ssc[:, k0:k0 + 1], scale=1.0, alpha=NEG_SLOPE)
                        pt = sp.tile([128, 512], bf16, tag="pt", bufs=3)
                        nc.scalar.activation(pt[:], et[:], AF.Exp)
                        wt0 = sp.tile([128, 512], bf16, tag="wt0", bufs=3)
                        nc.gpsimd.tensor_mul(wt0[:], pt[:], mf[:, 0, :])
                        ht = hts[k0 // 8][:, k0 % 8, :]
                        for m in range(4):
                            nc.tensor.matmul(
                                aggs[m][:], lhsT=ht[:, 128 * m:128 * (m + 1)],
                                rhs=wt0[:], start=(k0 == 0), stop=False)
                        k1 = 2 * kp + 1
                        u = sp.tile([128, 512], bf16, tag="u", bufs=3)
                        nc.vector.scalar_tensor_tensor(
                            u[:], F1[:, 512 * j:512 * (j + 1)],
                            E1[:, k1:k1 + 1], mf[:, 1, :],
                            op0=ALU.mult, op1=ALU.mult)
                        v2 = sp.tile([128, 512], bf16, tag="v", bufs=3)
                        nc.vector.scalar_tensor_tensor(
                            v2[:], F2[:, 512 * j:512 * (j + 1)],
                            E2[:, k1:k1 + 1], mf[:, 1, :],
                            op0=ALU.mult, op1=ALU.mult)
                        wt1 = sp.tile([128, 512], bf16, tag="wt", bufs=3)
                        nc.vector.tensor_tensor(wt1[:], u[:], v2[:], op=ALU.max)
                        ht = hts[k1 // 8][:, k1 % 8, :]
                        for m in range(4):
                            nc.tensor.matmul(
                                aggs[m][:], lhsT=ht[:, 128 * m:128 * (m + 1)],
                                rhs=wt1[:], start=False, stop=(k1 == KT - 1))
                        if kp == 0:
                            nc.vector.tensor_add(wsum[:], wt0[:], wt1[:])
                        else:
                            wpair = sp.tile([128, 512], bf16, tag="wpair",
                                            bufs=2)
                            nc.vector.tensor_add(wpair[:], wt0[:], wt1[:])
                            nc.vector.tensor_add(wsum[:], wsum[:], wpair[:])
                    den = pp.tile([1, 512], f32, tag="den")
                    nc.tensor.matmul(den[:], lhsT=ones_cb[:], rhs=wsum[:],
                                     start=True, stop=True)
                    denr = rp.tile([1, 512], f32, tag="row")
                    nc.vector.tensor_copy(denr[:], den[:])
                    invp = pp.tile([128, 512], f32, tag="invb")
                    nc.tensor.matmul(invp[:], lhsT=ones_r[:], rhs=denr[:],
                                     start=True, stop=True)
                    invs = rp.tile([128, 512], f32, tag="invs")
                    nc.vector.reciprocal_approx_fast(invs[:], invp[:])
                    for m in range(4):
                        write_out(j, m, aggs[m], invs)

            # ---------- GAT layer 1 ----------
            sdb1 = compute_sdb(wsd1, l0o, "sdb")
            F11, F21, E11, E21, sc1 = exp_factors(sdb1, s1g)
            hts1 = load_hres(h1g, nc.sync, "h")
            l1own = op_.tile([128, 4, NO], bf16, tag="l1own")

            def write_l1(j, m, agg, invs):
                tmp = sp.tile([128, 512], f32, tag="tmp", bufs=1)
                nc.vector.tensor_mul(tmp[:], agg[:], invs[:])
                nc.vector.tensor_scalar_add(
                    l1own[:, m, 512 * j:512 * (j + 1)], tmp[:], bg1f[:, m:m + 1])

            gat_loop(hts1, F11, F21, E11, E21, sdb1, sc1, 1, write_l1)

            # ---------- ssrc2 + h2 own -> AllGather (fly during attention) ----
            wsd2 = compute_wsd(g2, a2, "wsd2")
            ssrc_own(wsd2, l1own, s2o)
            nc.gpsimd.collective_compute(
                "AllGather", mybir.AluOpType.bypass,
                replica_groups=RG, ins=[s2o.opt()], outs=[s2g.opt()])
            for tp in range(4):
                st2 = sp.tile([128, 2, 512], bf16, tag="stg2", bufs=2)
                for ti in range(2):
                    t = 2 * tp + ti
                    ps = ppmm.tile([128, 512], f32, tag="mm")
                    for k2 in range(4):
                        nc.tensor.matmul(
                            ps[:], lhsT=l1own[:, k2, 128 * t:128 * (t + 1)],
                            rhs=g2T[:, k2, :], start=(k2 == 0), stop=(k2 == 3))
                    nc.vector.tensor_copy(st2[:, ti, :], ps[:])
                nc.sync.dma_start(
                    h2o_b[2 * tp:2 * tp + 2, :, :].rearrange("a p c -> p a c"),
                    st2[:])
            nc.gpsimd.collective_compute(
                "AllGather", mybir.AluOpType.bypass,
                replica_groups=RG, ins=[h2o_b.opt()], outs=[h2g.opt()])

            # ---------- attention ----------
            at16 = ap.tile([128, 4, 512], bf16, tag="at16")
            for qh in range(2):
                avs = [pp.tile([128, 512], f32, tag=f"agg{m}",
                               name=f"av{m}") for m in range(4)]
                esum = rp.tile([128, 512], bf16, tag="wsum")
                for kkp in range(KT // 2):
                    kk0 = 2 * kkp
                    cr = kk0 // 8
                    dl = kk0 % 8
                    ktile = lp.tile([128, 4, 256], bf16, tag="kst", bufs=3)
                    nc.sync.dma_start(
                        ktile[:], kg[cr, :, :, 128 * dl:128 * (dl + 2)]
                        .rearrange("a p c -> p a c"))
                    vtile = lp.tile([128, 2, 512], bf16, tag="vst", bufs=2)
                    nc.scalar.dma_start(
                        vtile[:], vg[cr, dl:dl + 2, :, :]
                        .rearrange("a p c -> p a c"))
                    ess = []
                    for ci in range(2):
                        kk = kk0 + ci
                        pscr = ppmm.tile([128, 512], f32, tag="mm")
                        for k2 in range(4):
                            nc.tensor.matmul(
                                pscr[:],
                                lhsT=ktile[:, k2, 128 * ci:128 * (ci + 1)],
                                rhs=q16[:, k2, 512 * qh:512 * (qh + 1)],
                                start=(k2 == 0), stop=(k2 == 3))
                        es = sp.tile([128, 512], bf16, tag="es", bufs=3)
                        nc.scalar.activation(es[:], pscr[:], AF.Exp, scale=SCL)
                        ess.append(es)
                        for m in range(4):
                            nc.tensor.matmul(
                                avs[m][:],
                                lhsT=vtile[:, ci, 128 * m:128 * (m + 1)],
                                rhs=es[:], start=(kk == 0),
                                stop=(kk == KT - 1))
                    if kkp == 0:
                        nc.vector.tensor_add(esum[:], ess[0][:], ess[1][:])
                    else:
                        epair = sp.tile([128, 512], bf16, tag="wpair", bufs=2)
                        nc.vector.tensor_add(epair[:], ess[0][:], ess[1][:])
                        nc.vector.tensor_add(esum[:], esum[:], epair[:])
                avden = pp.tile([1, 512], f32, tag="den")
                nc.tensor.matmul(avden[:], lhsT=ones_cb[:], rhs=esum[:],
                                 start=True, stop=True)
                denr = rp.tile([1, 512], f32, tag="row")
                nc.vector.tensor_copy(denr[:], avden[:])
                invp = pp.tile([128, 512], f32, tag="invb")
                nc.tensor.matmul(invp[:], lhsT=ones_r[:], rhs=denr[:],
                                 start=True, stop=True)
                invs = rp.tile([128, 512], f32, tag="invs")
                nc.vector.reciprocal_approx_fast(invs[:], invp[:])
                for m in range(4):
                    nc.vector.tensor_mul(at16[:, m, :], avs[m][:], invs[:])
                for m in range(4):
                    ps = ppmm.tile([128, 512], f32, tag="mm")
                    for k2 in range(4):
                        nc.tensor.matmul(
                            ps[:], lhsT=oT[:, k2, 128 * m:128 * (m + 1)],
                            rhs=at16[:, k2, :], start=(k2 == 0), stop=(k2 == 3))
                    stf = sp.tile([128, 512], f32, tag="stgf", bufs=2)
                    nc.vector.tensor_scalar_add(stf[:], ps[:], bof2[:, m:m + 1])
                    nc.scalar.dma_start(
                        out_g[m, :, 512 * qh:512 * (qh + 1)], stf[:])

            # ---------- GAT layer 2 ----------
            sdb2 = compute_sdb(wsd2, l1own, "sdb")
            F12, F22, E12, E22, sc2 = exp_factors(sdb2, s2g)
            hts2 = load_hres(h2g, nc.sync, "h")

            def write_l2(j, m, agg, invs):
                tmp = sp.tile([128, 512], f32, tag="tmp", bufs=1)
                nc.vector.tensor_mul(tmp[:], agg[:], invs[:])
                stf = sp.tile([128, 512], f32, tag="stgf", bufs=2)
                nc.vector.tensor_scalar_add(stf[:], tmp[:], bg2f[:, m:m + 1])
                nc.scalar.dma_start(
                    out_l[m, :, 512 * j:512 * (j + 1)], stf[:])

            gat_loop(hts2, F12, F22, E12, E22, sdb2, sc2, 2, write_l2)

    nc.finalize()
    return nc


def _prep_tables(src, dst):
    """Per-core, per-dst-half scatter tables for JIT mask construction.

    For dst half j of core c (512 dst nodes), edges are bucketed by
    (src chunk-pair kp = src//256, src partition p = src%128); the scatter
    writes count values at column (src//128 % 2)*512 + (dst - base) of a
    [128, 1024] tile covering src chunks 2kp, 2kp+1."""
    import ml_dtypes
    per = {0: [], 1: []}
    Wmax = [0, 0]
    for c in range(N_CORES):
        for j in range(2):
            lo = c * NO + 512 * j
            sel = (dst >= lo) & (dst < lo + 512)
            s = src[sel].astype(np.int64)
            dcol = (dst[sel] - lo).astype(np.int64)
            key = s * 512 + dcol
            uniq, counts = np.unique(key, return_counts=True)
            s_u = uniq // 512
            col = (uniq % 512) + 512 * ((s_u // 128) % 2)
            kp = s_u // 256
            p = s_u % 128
            bucket = kp * 128 + p
            order = np.argsort(bucket, kind="stable")
            bucket = bucket[order]
            col = col[order]
            counts = counts[order]
            bstart = np.r_[0, np.flatnonzero(np.diff(bucket)) + 1]
            sizes = np.diff(np.r_[bstart, bucket.size])
            slot = np.arange(bucket.size) - np.repeat(bstart, sizes)
            Wmax[j] = max(Wmax[j], int(sizes.max()) if sizes.size else 0)
            per[j].append((bucket, col, counts, slot))
    Ws = [max(2, (w + 1) // 2 * 2) for w in Wmax]
    tables = []
    for c in range(N_CORES):
        t = {}
        for j in range(2):
            W = Ws[j]
            bucket, col, counts, slot = per[j][c]
            sc_idx = np.full((128, KP * W), -1, np.int16)
            sc_cnt = np.zeros((128, KP * W), ml_dtypes.bfloat16)
            kp = bucket // 128
            p = bucket % 128
            flat = kp * W + slot
            sc_idx[p, flat] = col.astype(np.int16)
            sc_cnt[p, flat] = counts.astype(np.float32)
            t[f"sc_idx{j}"] = sc_idx
            t[f"sc_cnt{j}"] = sc_cnt
        tables.append(t)
    return Ws[0], Ws[1], tables


def kernel(**inputs):
    global LAST_EXEC_NS, _LAST_RES
    from concourse.bass_utils import run_bass_kernel_spmd

    f = lambda name: np.ascontiguousarray(np.asarray(inputs[name], np.float32))
    x_A, x_B = f("x_A"), f("x_B")
    eAB = np.asarray(inputs["edge_AB"]).astype(np.int64)
    eBA = np.asarray(inputs["edge_BA"]).astype(np.int64)

    src = np.concatenate([eAB[0], eBA[0] + NA, np.arange(N, dtype=np.int64)])
    dst = np.concatenate([eAB[1] + NA, eBA[1], np.arange(N, dtype=np.int64)])
    W0, W1, tables = _prep_tables(src, dst)

    if (W0, W1) not in _CACHE:
        _CACHE[(W0, W1)] = _build(W0, W1)
    nc = _CACHE[(W0, W1)]

    import ml_dtypes
    b16 = lambda a: np.ascontiguousarray(np.asarray(a, ml_dtypes.bfloat16))
    xT = np.ascontiguousarray(np.concatenate([x_A, x_B], 0).T)
    col = lambda name: f(name).reshape(-1, 1)
    WqkvT = f("Wqkv").T  # [H, 3H]
    shared = {
        "Wg1": b16(f("Wg1")), "Wg1_T": b16(f("Wg1").T),
        "A1": b16(np.stack([f("a_src1"), f("a_dst1")], 1)),
        "bg1": col("bg1"),
        "Wg2": b16(f("Wg2")), "Wg2_T": b16(f("Wg2").T),
        "A2": b16(np.stack([f("a_src2"), f("a_dst2")], 1)),
        "bg2": col("bg2"),
        "WqT": b16(WqkvT[:, 0:H]),
        "WkT": b16(WqkvT[:, H:2 * H]),
        "WvT": b16(WqkvT[:, 2 * H:3 * H]),
        "bq": col("bqkv")[0:H], "bk": col("bqkv")[H:2 * H],
        "bv": col("bqkv")[2 * H:3 * H],
        "WoT": b16(f("Wo").T), "bo": col("bo"),
    }
    WinA_T = b16(f("W_inA").T)
    WinB_T = b16(f("W_inB").T)
    Win2A_T = b16(f("W_in2A").T)
    Win2B_T = b16(f("W_in2B").T)
    in_maps = []
    for c in range(N_CORES):
        m = dict(shared)
        m["xoT"] = b16(xT[:, c * NO:(c + 1) * NO])
        if c < N_CORES // 2:
            m["win_o"] = WinA_T; m["bin_o"] = col("b_inA")
            m["win2_o"] = Win2A_T; m["bin2_o"] = col("b_in2A")
        else:
            m["win_o"] = WinB_T; m["bin_o"] = col("b_inB")
            m["win2_o"] = Win2B_T; m["bin2_o"] = col("b_in2B")
        m.update(tables[c])
        in_maps.append(m)

    if TRACE:
        _install_trace_hook()
    res = run_bass_kernel_spmd(nc, in_maps, list(range(N_CORES)),
                               trace=bool(TRACE))
    LAST_EXEC_NS = res.exec_time_ns
    _LAST_RES = res

    l_full = np.empty((N, H), np.float32)
    g_full = np.empty((N, H), np.float32)
    for c in range(N_CORES):
        r = res.results[c]
        l_full[c * NO:(c + 1) * NO] = r["out_l"].reshape(H, NO).T
        g_full[c * NO:(c + 1) * NO] = r["out_g"].reshape(H, NO).T
    z_A = np.concatenate([l_full[:NA], g_full[:NA]], 1)
    z_B = np.concatenate([l_full[NA:], g_full[NA:]], 1)
    return (z_A, z_B)


# revision 32
# speedup vs baseline: 1.6597x; 1.6597x over previous
"""Trainium2 Bass kernel for nn_GAT_T (2x GATConv + dense self-attention hybrid).

Sharding across 8 NeuronCores: core c owns nodes [1024c, 1024(c+1)).
 - GATConv: dst-node sharded, dense masked-softmax formulation. Per-layer
   h (all nodes) comes from an own-slice matmul + AllGather; the adjacency
   mask is built just-in-time inside the GAT loop by gpsimd local_scatter
   (no DRAM round trip). The LeakyReLU+exp is factored exactly as
   exp(lrelu(ss+sd)) = max(e^ss * e^sd, e^{.2ss} * e^{.2sd}) so the inner
   loop is 3 fused DVE ops (no ScalarE).
 - Dense NxN attention: query-row sharded; K/V own-slice + AllGather during
   GAT1, streamed in 512KB tiles during the attention phase.
 - Collectives (order): ssrc1 (4KB), h1 (1MB/core), k, v during prologue/
   GAT1; ssrc2 (4KB), h2 (1MB/core) during attention.
Heavy matmuls in bf16 with fp32 PSUM accumulation. Softmax computed without
max-subtraction (logits are O(+-6); mathematically identical).
"""

import numpy as np

NA, NB = 4096, 4096
N = NA + NB
IN, H = 256, 512
N_CORES = 8
NO = N // N_CORES      # 1024 nodes per core
KT = N // 128          # 64 src chunks
KP = KT // 2           # 32 src chunk-pairs
NEG_SLOPE = 0.2

TRACE = False
LAST_EXEC_NS = None
_LAST_RES = None
_CACHE = {}


def _install_trace_hook():
    import sys, types
    if "antenv.axon_hooks" in sys.modules:
        return
    try:
        mod = types.ModuleType("antenv.axon_hooks")
        mod._hook = None
        mod.set_axon_ntff_profile_hook = lambda h: setattr(mod, "_hook", h)
        mod.get_axon_ntff_profile_hook = lambda: mod._hook
        sys.modules["antenv.axon_hooks"] = mod
        from trn_agent_boot.trn_boot import _ntff_profile_via_ctypes
        mod.set_axon_ntff_profile_hook(
            _ntff_profile_via_ctypes("/opt/axon/libaxon_pjrt.so"))
    except Exception:
        pass


def _build(W0, W1):
    import concourse.bacc as bacc
    import concourse.mybir as mybir
    import concourse.tile as tile

    f32 = mybir.dt.float32
    bf16 = mybir.dt.bfloat16
    i16 = mybir.dt.int16
    AF = mybir.ActivationFunctionType
    ALU = mybir.AluOpType

    nc = bacc.Bacc("TRN2", target_bir_lowering=False, debug=False,
                   num_devices=N_CORES)

    def inp(name, shape, dt=f32):
        return nc.dram_tensor(name, shape, dt, kind="ExternalInput")

    # weights arrive pre-cast to bf16 from the host so every load is a plain
    # HWDGE DMA (keeps the gpsimd queue free for the barrier + cc triggers)
    xoT = inp("xoT", [IN, NO], bf16)
    xTb = inp("xTb", [IN, N], bf16)
    win_o = inp("win_o", [IN, H], bf16); win2_o = inp("win2_o", [IN, H], bf16)
    bin_o = inp("bin_o", [H, 1]); bin2_o = inp("bin2_o", [H, 1])
    WcA = inp("WcA", [IN, H], bf16); WcB = inp("WcB", [IN, H], bf16)
    SV = inp("SV", [IN, 2], bf16)
    bh1r = inp("bh1r", [2, H]); c0r = inp("c0r", [1, 2])
    WSD1 = inp("WSD1", [H, 2], bf16)
    WSD2 = inp("WSD2", [H, 2], bf16)
    Wg2_T = inp("Wg2_T", [H, H], bf16)
    bg1 = inp("bg1", [H, 1]); bg2 = inp("bg2", [H, 1])
    WqT = inp("WqT", [H, H], bf16); WkT = inp("WkT", [H, H], bf16)
    WvT = inp("WvT", [H, H], bf16)
    bq = inp("bq", [H, 1]); bk = inp("bk", [H, 1]); bv = inp("bv", [H, 1])
    WoT = inp("WoT", [H, H], bf16); bo = inp("bo", [H, 1])
    sc_idx0 = inp("sc_idx0", [128, KP * W0], i16)
    sc_cnt0 = inp("sc_cnt0", [128, KP * W0], bf16)
    sc_idx1 = inp("sc_idx1", [128, KP * W1], i16)
    sc_cnt1 = inp("sc_cnt1", [128, KP * W1], bf16)

    out_l = nc.dram_tensor("out_l", [4, 128, NO], f32, kind="ExternalOutput")
    out_g = nc.dram_tensor("out_g", [4, 128, NO], f32, kind="ExternalOutput")

    RG = [list(range(N_CORES))]
    SCL = 1.0 / float(np.sqrt(H))

    with tile.TileContext(nc) as tc:
        with (
            tc.tile_pool(name="wp", bufs=1) as wp,
            tc.tile_pool(name="apool", bufs=1) as ap,
            tc.tile_pool(name="own", bufs=1) as op_,
            tc.tile_pool(name="hp", bufs=1) as hp,
            tc.tile_pool(name="sp", bufs=3) as sp,
            tc.tile_pool(name="rp", bufs=2) as rp,
            tc.tile_pool(name="lp", bufs=3) as lp,
            tc.tile_pool(name="pp", bufs=1, space="PSUM") as pp,
            tc.tile_pool(name="ppmm", bufs=2, space="PSUM") as ppmm,
            tc.tile_pool(name="dram", bufs=1, space="DRAM") as dp,
        ):
            # ---------- load weights (HWDGE; already bf16) ----------
            def w16(dram, rows, cols, tag):
                t = wp.tile([128, rows // 128, cols], bf16, tag=tag)
                nc.sync.dma_start(
                    t[:], dram[:].rearrange("(a p) c -> p a c", p=128))
                return t

            def bias32(dram, tag):
                t = wp.tile([128, H // 128], f32, tag=tag)
                nc.scalar.dma_start(
                    t[:], dram[:].rearrange("(a p) one -> p (a one)", p=128))
                return t

            # own x first (scalar queue), prologue-critical weights (sync)
            xo = lp.tile([128, 2, NO], bf16, tag="xo", bufs=1)
            nc.scalar.dma_start(
                xo[:], xoT[:].rearrange("(a p) c -> p a c", p=128))
            wo_ = w16(win_o, IN, H, "wo_")
            bof_ = bias32(bin_o, "bof_")
            wsd1 = w16(WSD1, H, 2, "wsd1")
            wcA = w16(WcA, IN, H, "wcA")
            wcB = w16(WcB, IN, H, "wcB")
            svt = w16(SV, IN, 2, "svt")
            bg1f = bias32(bg1, "bg1f")
            c0t = wp.tile([1, 2], f32, tag="c0t")
            nc.scalar.dma_start(c0t[:], c0r[:])
            bh1s = wp.tile([1, 2, H], f32, tag="bh1s")
            nc.scalar.dma_start(
                bh1s[:], bh1r[:].rearrange("(o t) c -> o t c", o=1))
            w2o_ = w16(win2_o, IN, H, "w2o_")
            b2of_ = bias32(bin2_o, "b2of_")
            qT = w16(WqT, H, H, "qT"); bqf = bias32(bq, "bqf")
            kT = w16(WkT, H, H, "kT"); bkf = bias32(bk, "bkf")
            vT = w16(WvT, H, H, "vT")

            ones_r = wp.tile([1, 128], f32, tag="ones_r")
            nc.vector.memset(ones_r[:], 1.0)
            ones_cb = wp.tile([128, 1], bf16, tag="ones_cb")
            nc.vector.memset(ones_cb[:], 1.0)

            sci = [wp.tile([128, KP * W0], i16, tag="sci0", name="sci0"),
                   wp.tile([128, KP * W1], i16, tag="sci1", name="sci1")]
            scc = [wp.tile([128, KP * W0], bf16, tag="scc0", name="scc0"),
                   wp.tile([128, KP * W1], bf16, tag="scc1", name="scc1")]
            nc.scalar.dma_start(sci[0][:], sc_idx0[:])
            nc.scalar.dma_start(scc[0][:], sc_cnt0[:])
            nc.scalar.dma_start(sci[1][:], sc_idx1[:])
            nc.scalar.dma_start(scc[1][:], sc_cnt1[:])

            # bv as a broadcast row [128, 512] f32
            bvrow = wp.tile([1, H], f32, tag="bvrow")
            nc.scalar.dma_start(bvrow[:], bv[:].rearrange("f o -> o f"))

            # ---------- internal DRAM ----------
            s1_stage = dp.tile([1, N], f32, tag="s1st")
            ko_b = dp.tile([4, 128, NO], bf16, tag="kob")
            kg = dp.tile([N_CORES, 4, 128, NO], bf16, tag="kg",
                         addr_space="Shared")
            vo_b = dp.tile([8, 128, 512], bf16, tag="vob")
            vg = dp.tile([N_CORES, 8, 128, 512], bf16, tag="vg",
                         addr_space="Shared")
            s2o = dp.tile([1, NO], f32, tag="s2o")
            s2g = dp.tile([N_CORES, 1, NO], f32, tag="s2g", addr_space="Shared")
            h2o_b = dp.tile([8, 128, 512], bf16, tag="h2ob")
            h2g = dp.tile([N_CORES, 8, 128, 512], bf16, tag="h2g",
                          addr_space="Shared")

            # ---------- prologue: own input linears ----------
            l0o = op_.tile([128, 4, NO], bf16, tag="l0o")
            g0o = op_.tile([128, 4, NO], bf16, tag="g0o")
            for n2 in range(2):
                for m in range(4):
                    ps = ppmm.tile([128, 512], f32, tag="mm")
                    for k2 in range(2):
                        nc.tensor.matmul(
                            ps[:], lhsT=wo_[:, k2, 128 * m:128 * (m + 1)],
                            rhs=xo[:, k2, 512 * n2:512 * (n2 + 1)],
                            start=(k2 == 0), stop=(k2 == 1))
                    nc.vector.tensor_scalar_add(
                        l0o[:, m, 512 * n2:512 * (n2 + 1)], ps[:], bof_[:, m:m + 1])

            def ssrc_own(wsd, lown, s_dram):
                """own-node ssrc rows -> s_dram [1, NO] f32."""
                for n2 in range(2):
                    psd = pp.tile([1, 512], f32, tag="den")
                    for k2 in range(4):
                        nc.tensor.matmul(
                            psd[:], lhsT=wsd[:, k2, 0:1],
                            rhs=lown[:, k2, 512 * n2:512 * (n2 + 1)],
                            start=(k2 == 0), stop=(k2 == 3))
                    row = rp.tile([1, 512], f32, tag="row")
                    nc.vector.tensor_copy(row[:], psd[:])
                    nc.sync.dma_start(
                        s_dram[:, 512 * n2:512 * (n2 + 1)], row[:])


            # ---------- g0o, q, k, v own + AllGather k/v ----------
            for n2 in range(2):
                for m in range(4):
                    ps = ppmm.tile([128, 512], f32, tag="mm")
                    for k2 in range(2):
                        nc.tensor.matmul(
                            ps[:], lhsT=w2o_[:, k2, 128 * m:128 * (m + 1)],
                            rhs=xo[:, k2, 512 * n2:512 * (n2 + 1)],
                            start=(k2 == 0), stop=(k2 == 1))
                    nc.vector.tensor_scalar_add(
                        g0o[:, m, 512 * n2:512 * (n2 + 1)], ps[:], b2of_[:, m:m + 1])

            q16 = ap.tile([128, 4, NO], bf16, tag="q16")
            for n2 in range(2):
                for m in range(4):
                    ps = ppmm.tile([128, 512], f32, tag="mm")
                    for k2 in range(4):
                        nc.tensor.matmul(
                            ps[:], lhsT=qT[:, k2, 128 * m:128 * (m + 1)],
                            rhs=g0o[:, k2, 512 * n2:512 * (n2 + 1)],
                            start=(k2 == 0), stop=(k2 == 3))
                    nc.vector.tensor_scalar_add(
                        q16[:, m, 512 * n2:512 * (n2 + 1)], ps[:], bqf[:, m:m + 1])

            for n2 in range(2):
                for mp in range(2):
                    st2 = sp.tile([128, 2, 512], bf16, tag="stg2", bufs=2)
                    for mi in range(2):
                        m = 2 * mp + mi
                        ps = ppmm.tile([128, 512], f32, tag="mm")
                        for k2 in range(4):
                            nc.tensor.matmul(
                                ps[:], lhsT=kT[:, k2, 128 * m:128 * (m + 1)],
                                rhs=g0o[:, k2, 512 * n2:512 * (n2 + 1)],
                                start=(k2 == 0), stop=(k2 == 3))
                        nc.vector.tensor_scalar_add(
                            st2[:, mi, :], ps[:], bkf[:, m:m + 1])
                    nc.sync.dma_start(
                        ko_b[2 * mp:2 * mp + 2, :, 512 * n2:512 * (n2 + 1)]
                        .rearrange("a p c -> p a c"), st2[:])

            # bv broadcast [128, 512]
            pb = ppmm.tile([128, H], f32, tag="mm")
            nc.tensor.matmul(pb[:], lhsT=ones_r[:], rhs=bvrow[:], start=True,
                             stop=True)
            bvb = ap.tile([128, H], f32, tag="bvb")
            nc.vector.tensor_copy(bvb[:], pb[:])
            for tp in range(4):
                st2 = sp.tile([128, 2, 512], bf16, tag="stg2", bufs=2)
                for ti in range(2):
                    t = 2 * tp + ti
                    ps = ppmm.tile([128, 512], f32, tag="mm")
                    for k2 in range(4):
                        nc.tensor.matmul(
                            ps[:], lhsT=g0o[:, k2, 128 * t:128 * (t + 1)],
                            rhs=vT[:, k2, :], start=(k2 == 0), stop=(k2 == 3))
                    nc.vector.tensor_add(st2[:, ti, :], ps[:], bvb[:])
                nc.sync.dma_start(
                    vo_b[2 * tp:2 * tp + 2, :, :].rearrange("a p c -> p a c"),
                    st2[:])
            nc.gpsimd.collective_compute(
                "AllGather", mybir.AluOpType.bypass,
                replica_groups=RG, ins=[ko_b.opt()], outs=[kg.opt()])
            nc.gpsimd.collective_compute(
                "AllGather", mybir.AluOpType.bypass,
                replica_groups=RG, ins=[vo_b.opt()], outs=[vg.opt()])

            # layer-2 / attention-epilogue weights, reusing layer-1 slots
            # (issued after the collective triggers so the WAR waits cannot
            # delay them; the donors' last reads are all in the prologue)
            g2T = w16(Wg2_T, H, H, "g1T")
            wsd2 = w16(WSD2, H, 2, "wsd1")
            bg2f = bias32(bg2, "bg2f")
            oT = w16(WoT, H, H, "kT")
            bof2 = bias32(bo, "bof2")

            # ---------- fused stage-0 full: h1 = x @ Wc + bh1 (all nodes) ----
            # Wc = W_in^T Wg1^T is host-fused, so h1 for every node comes
            # straight from x with no l0 staging; runs during the startup
            # barrier + k/v gathers. ssrc1 = x @ SV + c0 likewise.
            hts1 = [hp.tile([128, 8, 512], bf16, tag=f"h{c}", name=f"h1_{c}")
                    for c in range(N_CORES)]
            bh1b = []
            for t in range(2):
                pbh = ppmm.tile([128, H], f32, tag="mm")
                nc.tensor.matmul(pbh[:], lhsT=ones_r[:], rhs=bh1s[0:1, t, :],
                                 start=True, stop=True)
                bb = ap.tile([128, H], bf16, tag=f"bh1b{t}", name=f"bh1b{t}")
                nc.vector.tensor_copy(bb[:], pbh[:])
                bh1b.append(bb)
            for n16 in range(16):
                ta = 0 if n16 < 8 else 1
                wc = wcA if n16 < 8 else wcB
                xq = lp.tile([128, 2, 512], bf16, tag="xq", bufs=2)
                nc.scalar.dma_start(
                    xq[:], xTb[:, 512 * n16:512 * (n16 + 1)]
                    .rearrange("(a p) c -> p a c", p=128))
                for t in range(4):
                    kk = 4 * n16 + t
                    ps = ppmm.tile([128, 512], f32, tag="mm")
                    for k2 in range(2):
                        nc.tensor.matmul(
                            ps[:], lhsT=xq[:, k2, 128 * t:128 * (t + 1)],
                            rhs=wc[:, k2, :], start=(k2 == 0), stop=(k2 == 1))
                    nc.vector.tensor_add(
                        hts1[kk // 8][:, kk % 8, :], ps[:], bh1b[ta][:])
                psd = pp.tile([1, 512], f32, tag="den")
                for k2 in range(2):
                    nc.tensor.matmul(
                        psd[:], lhsT=svt[:, k2, ta:ta + 1], rhs=xq[:, k2, :],
                        start=(k2 == 0), stop=(k2 == 1))
                row = rp.tile([1, 512], f32, tag="row")
                nc.vector.tensor_scalar_add(row[:], psd[:], c0t[0:1, ta:ta + 1])
                nc.sync.dma_start(
                    s1_stage[:, 512 * n16:512 * (n16 + 1)], row[:])
            sc1 = ap.tile([128, KT], f32, tag="sc", name="sc1")
            nc.scalar.dma_start(
                sc1[:], s1_stage[0:1, :].rearrange("o (t p) -> p (o t)", p=128))

            # ---------- helpers ----------
            def compute_sdb(wsd, lown, tag):
                """sdst over own nodes, broadcast to [128, NO] f32."""
                sdb = ap.tile([128, NO], bf16, tag=tag)
                for n2 in range(2):
                    psd = pp.tile([1, 512], f32, tag="den")
                    for k2 in range(4):
                        nc.tensor.matmul(
                            psd[:], lhsT=wsd[:, k2, 1:2],
                            rhs=lown[:, k2, 512 * n2:512 * (n2 + 1)],
                            start=(k2 == 0), stop=(k2 == 3))
                    row = rp.tile([1, 512], f32, tag="row")
                    nc.vector.tensor_copy(row[:], psd[:])
                    psb = ppmm.tile([128, 512], f32, tag="mm")
                    nc.tensor.matmul(psb[:], lhsT=ones_r[:], rhs=row[:],
                                     start=True, stop=True)
                    nc.vector.tensor_copy(sdb[:, 512 * n2:512 * (n2 + 1)], psb[:])
                return sdb

            def exp_factors(sdb, sc):
                """F1/F2 [128, NO] bf16 (exp of sdst, exp of .2 sdst) and
                E1/E2 [128, KT] f32 (exp of ssrc)."""
                F1 = ap.tile([128, NO], bf16, tag="F1", name="F1")
                F2 = ap.tile([128, NO], bf16, tag="F2", name="F2")
                nc.scalar.activation(F1[:], sdb[:], AF.Exp)
                nc.scalar.activation(F2[:], sdb[:], AF.Exp, scale=NEG_SLOPE)
                E1 = ap.tile([128, KT], f32, tag="E1", name="E1")
                E2 = ap.tile([128, KT], f32, tag="E2", name="E2")
                nc.scalar.activation(E1[:], sc[:], AF.Exp)
                nc.scalar.activation(E2[:], sc[:], AF.Exp, scale=NEG_SLOPE)
                return F1, F2, E1, E2

            def load_hres(h_g, queue, tag):
                hts = []
                for c in range(N_CORES):
                    ht = hp.tile([128, 8, 512], bf16, tag=f"{tag}{c}")
                    queue.dma_start(
                        ht[:], h_g[c, :, :, :].rearrange("a p c -> p a c"))
                    hts.append(ht)
                return hts

            def gat_loop(hts, F1, F2, E1, E2, sdb, ssc, li, write_out):
                # per chunk-pair: ki=0 attention weights on ScalarE
                # (Prelu+Exp) with the mask multiply on gpsimd; ki=1 on DVE
                # (two fused stt + max). Balances all four engines under PE.
                for j in range(2):
                    Wj = W0 if j == 0 else W1
                    aggs = [pp.tile([128, 512], f32, tag=f"agg{m}",
                                    name=f"agg{li}{m}") for m in range(4)]
                    wsum = rp.tile([128, 512], bf16, tag="wsum")
                    for kp in range(KP):
                        mf = sp.tile([128, 2, 512], bf16, tag="mask", bufs=3)
                        nc.gpsimd.local_scatter(
                            out_ap=mf[:], data_ap=scc[j][:, kp * Wj:(kp + 1) * Wj],
                            idxs_ap=sci[j][:, kp * Wj:(kp + 1) * Wj],
                            channels=128, num_elems=NO, num_idxs=Wj)
                        k0 = 2 * kp
                        et = sp.tile([128, 512], f32, tag="et", bufs=2)
                        nc.scalar.activation(
                            et[:], sdb[:, 512 * j:512 * (j + 1)], AF.Prelu,
                            bias=ssc[:, k0:k0 + 1], scale=1.0, alpha=NEG_SLOPE)
                        pt = sp.tile([128, 512], bf16, tag="pt", bufs=2)
                        nc.scalar.activation(pt[:], et[:], AF.Exp)
                        wt0 = sp.tile([128, 512], bf16, tag="wt0", bufs=3)
                        nc.vector.tensor_mul(wt0[:], pt[:], mf[:, 0, :])
                        ht = hts[k0 // 8][:, k0 % 8, :]
                        for m in range(4):
                            nc.tensor.matmul(
                                aggs[m][:], lhsT=ht[:, 128 * m:128 * (m + 1)],
                                rhs=wt0[:], start=(k0 == 0), stop=False)
                        k1 = 2 * kp + 1
                        u = sp.tile([128, 512], bf16, tag="u", bufs=2)
                        nc.vector.scalar_tensor_tensor(
                            u[:], F1[:, 512 * j:512 * (j + 1)],
                            E1[:, k1:k1 + 1], mf[:, 1, :],
                            op0=ALU.mult, op1=ALU.mult)
                        v2 = sp.tile([128, 512], bf16, tag="v", bufs=2)
                        nc.vector.scalar_tensor_tensor(
                            v2[:], F2[:, 512 * j:512 * (j + 1)],
                            E2[:, k1:k1 + 1], mf[:, 1, :],
                            op0=ALU.mult, op1=ALU.mult)
                        wt1 = sp.tile([128, 512], bf16, tag="wt", bufs=3)
                        nc.vector.tensor_tensor(wt1[:], u[:], v2[:], op=ALU.max)
                        ht = hts[k1 // 8][:, k1 % 8, :]
                        for m in range(4):
                            nc.tensor.matmul(
                                aggs[m][:], lhsT=ht[:, 128 * m:128 * (m + 1)],
                                rhs=wt1[:], start=False, stop=(k1 == KT - 1))
                        if kp == 0:
                            nc.vector.tensor_add(wsum[:], wt0[:], wt1[:])
                        else:
                            wpair = sp.tile([128, 512], bf16, tag="wpair",
                                            bufs=2)
                            nc.vector.tensor_add(wpair[:], wt0[:], wt1[:])
                            nc.vector.tensor_add(wsum[:], wsum[:], wpair[:])
                    den = pp.tile([1, 512], f32, tag="den")
                    nc.tensor.matmul(den[:], lhsT=ones_cb[:], rhs=wsum[:],
                                     start=True, stop=True)
                    denr = rp.tile([1, 512], f32, tag="row")
                    nc.vector.tensor_copy(denr[:], den[:])
                    invp = pp.tile([128, 512], f32, tag="invb")
                    nc.tensor.matmul(invp[:], lhsT=ones_r[:], rhs=denr[:],
                                     start=True, stop=True)
                    invs = rp.tile([128, 512], f32, tag="invs", bufs=1)
                    nc.vector.reciprocal_approx_fast(invs[:], invp[:])
                    for m in range(4):
                        write_out(j, m, aggs[m], invs)

            # ---------- GAT layer 1 ----------
            sdb1 = compute_sdb(wsd1, l0o, "sdb")
            F11, F21, E11, E21 = exp_factors(sdb1, sc1)
            l1own = op_.tile([128, 4, NO], bf16, tag="l1own")

            def write_l1(j, m, agg, invs):
                tmp = sp.tile([128, 512], f32, tag="tmp", bufs=1)
                nc.vector.tensor_mul(tmp[:], agg[:], invs[:])
                nc.vector.tensor_scalar_add(
                    l1own[:, m, 512 * j:512 * (j + 1)], tmp[:], bg1f[:, m:m + 1])

            gat_loop(hts1, F11, F21, E11, E21, sdb1, sc1, 1, write_l1)

            # ---------- ssrc2 + h2 own -> AllGather (fly during attention) ----
            ssrc_own(wsd2, l1own, s2o)
            nc.gpsimd.collective_compute(
                "AllGather", mybir.AluOpType.bypass,
                replica_groups=RG, ins=[s2o.opt()], outs=[s2g.opt()])
            for tp in range(4):
                st2 = sp.tile([128, 2, 512], bf16, tag="stg2", bufs=2)
                for ti in range(2):
                    t = 2 * tp + ti
                    ps = ppmm.tile([128, 512], f32, tag="mm")
                    for k2 in range(4):
                        nc.tensor.matmul(
                            ps[:], lhsT=l1own[:, k2, 128 * t:128 * (t + 1)],
                            rhs=g2T[:, k2, :], start=(k2 == 0), stop=(k2 == 3))
                    nc.vector.tensor_copy(st2[:, ti, :], ps[:])
                nc.sync.dma_start(
                    h2o_b[2 * tp:2 * tp + 2, :, :].rearrange("a p c -> p a c"),
                    st2[:])
            nc.gpsimd.collective_compute(
                "AllGather", mybir.AluOpType.bypass,
                replica_groups=RG, ins=[h2o_b.opt()], outs=[h2g.opt()])

            # ---------- attention ----------
            at16 = ap.tile([128, 4, 512], bf16, tag="at16")
            for qh in range(2):
                avs = [pp.tile([128, 512], f32, tag=f"agg{m}",
                               name=f"av{m}") for m in range(4)]
                esum = rp.tile([128, 512], bf16, tag="wsum")
                for kkp in range(KT // 2):
                    kk0 = 2 * kkp
                    cr = kk0 // 8
                    dl = kk0 % 8
                    ktile = lp.tile([128, 4, 256], bf16, tag="kst", bufs=2)
                    nc.sync.dma_start(
                        ktile[:], kg[cr, :, :, 128 * dl:128 * (dl + 2)]
                        .rearrange("a p c -> p a c"))
                    vtile = lp.tile([128, 2, 512], bf16, tag="vst", bufs=2)
                    nc.scalar.dma_start(
                        vtile[:], vg[cr, dl:dl + 2, :, :]
                        .rearrange("a p c -> p a c"))
                    ess = []
                    for ci in range(2):
                        kk = kk0 + ci
                        pscr = ppmm.tile([128, 512], f32, tag="mm")
                        for k2 in range(4):
                            nc.tensor.matmul(
                                pscr[:],
                                lhsT=ktile[:, k2, 128 * ci:128 * (ci + 1)],
                                rhs=q16[:, k2, 512 * qh:512 * (qh + 1)],
                                start=(k2 == 0), stop=(k2 == 3))
                        es = sp.tile([128, 512], bf16, tag="es", bufs=2)
                        nc.scalar.activation(es[:], pscr[:], AF.Exp, scale=SCL)
                        ess.append(es)
                        for m in range(4):
                            nc.tensor.matmul(
                                avs[m][:],
                                lhsT=vtile[:, ci, 128 * m:128 * (m + 1)],
                                rhs=es[:], start=(kk == 0),
                                stop=(kk == KT - 1))
                    if kkp == 0:
                        nc.vector.tensor_add(esum[:], ess[0][:], ess[1][:])
                    else:
                        epair = sp.tile([128, 512], bf16, tag="wpair", bufs=2)
                        nc.vector.tensor_add(epair[:], ess[0][:], ess[1][:])
                        nc.vector.tensor_add(esum[:], esum[:], epair[:])
                avden = pp.tile([1, 512], f32, tag="den")
                nc.tensor.matmul(avden[:], lhsT=ones_cb[:], rhs=esum[:],
                                 start=True, stop=True)
                denr = rp.tile([1, 512], f32, tag="row")
                nc.vector.tensor_copy(denr[:], avden[:])
                invp = pp.tile([128, 512], f32, tag="invb")
                nc.tensor.matmul(invp[:], lhsT=ones_r[:], rhs=denr[:],
                                 start=True, stop=True)
                invs = rp.tile([128, 512], f32, tag="invs", bufs=1)
                nc.vector.reciprocal_approx_fast(invs[:], invp[:])
                for m in range(4):
                    nc.vector.tensor_mul(at16[:, m, :], avs[m][:], invs[:])
                for m in range(4):
                    ps = ppmm.tile([128, 512], f32, tag="mm")
                    for k2 in range(4):
                        nc.tensor.matmul(
                            ps[:], lhsT=oT[:, k2, 128 * m:128 * (m + 1)],
                            rhs=at16[:, k2, :], start=(k2 == 0), stop=(k2 == 3))
                    stf = sp.tile([128, 512], f32, tag="stgf", bufs=2)
                    nc.vector.tensor_scalar_add(stf[:], ps[:], bof2[:, m:m + 1])
                    nc.scalar.dma_start(
                        out_g[m, :, 512 * qh:512 * (qh + 1)], stf[:])

            # ---------- GAT layer 2 ----------
            sdb2 = compute_sdb(wsd2, l1own, "sdb")
            sc2 = ap.tile([128, KT], f32, tag="sc", name="sc2")
            nc.scalar.dma_start(
                sc2[:], s2g[:].rearrange("c o (a p) -> p (c a o)", p=128))
            F12, F22, E12, E22 = exp_factors(sdb2, sc2)
            hts2 = load_hres(h2g, nc.sync, "h")

            def write_l2(j, m, agg, invs):
                tmp = sp.tile([128, 512], f32, tag="tmp", bufs=1)
                nc.vector.tensor_mul(tmp[:], agg[:], invs[:])
                stf = sp.tile([128, 512], f32, tag="stgf", bufs=2)
                nc.vector.tensor_scalar_add(stf[:], tmp[:], bg2f[:, m:m + 1])
                nc.scalar.dma_start(
                    out_l[m, :, 512 * j:512 * (j + 1)], stf[:])

            gat_loop(hts2, F12, F22, E12, E22, sdb2, sc2, 2, write_l2)

    nc.finalize()
    return nc


def _prep_tables(src, dst):
    """Per-core, per-dst-half scatter tables for JIT mask construction.

    For dst half j of core c (512 dst nodes), edges are bucketed by
    (src chunk-pair kp = src//256, src partition p = src%128); the scatter
    writes count values at column (src//128 % 2)*512 + (dst - base) of a
    [128, 1024] tile covering src chunks 2kp, 2kp+1."""
    import ml_dtypes
    per = {0: [], 1: []}
    Wmax = [0, 0]
    for c in range(N_CORES):
        for j in range(2):
            lo = c * NO + 512 * j
            sel = (dst >= lo) & (dst < lo + 512)
            s = src[sel].astype(np.int64)
            dcol = (dst[sel] - lo).astype(np.int64)
            key = s * 512 + dcol
            uniq, counts = np.unique(key, return_counts=True)
            s_u = uniq // 512
            col = (uniq % 512) + 512 * ((s_u // 128) % 2)
            kp = s_u // 256
            p = s_u % 128
            bucket = kp * 128 + p
            order = np.argsort(bucket, kind="stable")
            bucket = bucket[order]
            col = col[order]
            counts = counts[order]
            bstart = np.r_[0, np.flatnonzero(np.diff(bucket)) + 1]
            sizes = np.diff(np.r_[bstart, bucket.size])
            slot = np.arange(bucket.size) - np.repeat(bstart, sizes)
            Wmax[j] = max(Wmax[j], int(sizes.max()) if sizes.size else 0)
            per[j].append((bucket, col, counts, slot))
    Ws = [max(2, (w + 1) // 2 * 2) for w in Wmax]
    tables = []
    for c in range(N_CORES):
        t = {}
        for j in range(2):
            W = Ws[j]
            bucket, col, counts, slot = per[j][c]
            sc_idx = np.full((128, KP * W), -1, np.int16)
            sc_cnt = np.zeros((128, KP * W), ml_dtypes.bfloat16)
            kp = bucket // 128
            p = bucket % 128
            flat = kp * W + slot
            sc_idx[p, flat] = col.astype(np.int16)
            sc_cnt[p, flat] = counts.astype(np.float32)
            t[f"sc_idx{j}"] = sc_idx
            t[f"sc_cnt{j}"] = sc_cnt
        tables.append(t)
    return Ws[0], Ws[1], tables


def kernel(**inputs):
    global LAST_EXEC_NS, _LAST_RES
    from concourse.bass_utils import run_bass_kernel_spmd

    f = lambda name: np.ascontiguousarray(np.asarray(inputs[name], np.float32))
    x_A, x_B = f("x_A"), f("x_B")
    eAB = np.asarray(inputs["edge_AB"]).astype(np.int64)
    eBA = np.asarray(inputs["edge_BA"]).astype(np.int64)

    src = np.concatenate([eAB[0], eBA[0] + NA, np.arange(N, dtype=np.int64)])
    dst = np.concatenate([eAB[1] + NA, eBA[1], np.arange(N, dtype=np.int64)])
    W0, W1, tables = _prep_tables(src, dst)

    if (W0, W1) not in _CACHE:
        _CACHE[(W0, W1)] = _build(W0, W1)
    nc = _CACHE[(W0, W1)]

    import ml_dtypes
    b16 = lambda a: np.ascontiguousarray(np.asarray(a, ml_dtypes.bfloat16))
    xT = np.ascontiguousarray(np.concatenate([x_A, x_B], 0).T)
    col = lambda name: f(name).reshape(-1, 1)
    WqkvT = f("Wqkv").T  # [H, 3H]
    # host-fused stage-0 weights (f64 for a single rounding step)
    Wg1d = np.float64(f("Wg1"))
    WcA_ = np.float64(f("W_inA")).T @ Wg1d.T
    WcB_ = np.float64(f("W_inB")).T @ Wg1d.T
    bh1A = Wg1d @ np.float64(f("b_inA"))
    bh1B = Wg1d @ np.float64(f("b_inB"))
    asrc1 = np.float64(f("a_src1"))
    shared = {
        "xTb": b16(xT),
        "WcA": b16(WcA_), "WcB": b16(WcB_),
        "SV": b16(np.stack([WcA_ @ asrc1, WcB_ @ asrc1], 1)),
        "bh1r": np.ascontiguousarray(
            np.stack([bh1A, bh1B], 0).astype(np.float32)),
        "c0r": np.array([[bh1A @ asrc1, bh1B @ asrc1]], np.float32),
        "WSD1": b16(np.stack([Wg1d.T @ asrc1,
                              Wg1d.T @ np.float64(f("a_dst1"))], 1)),
        "WSD2": b16(np.stack([np.float64(f("Wg2")).T @ np.float64(f("a_src2")),
                              np.float64(f("Wg2")).T @ np.float64(f("a_dst2"))],
                             1)),
        "bg1": col("bg1"),
        "Wg2_T": b16(f("Wg2").T),
        "bg2": col("bg2"),
        "WqT": b16(WqkvT[:, 0:H]),
        "WkT": b16(WqkvT[:, H:2 * H]),
        "WvT": b16(WqkvT[:, 2 * H:3 * H]),
        "bq": col("bqkv")[0:H], "bk": col("bqkv")[H:2 * H],
        "bv": col("bqkv")[2 * H:3 * H],
        "WoT": b16(f("Wo").T), "bo": col("bo"),
    }
    WinA_T = b16(f("W_inA").T)
    WinB_T = b16(f("W_inB").T)
    Win2A_T = b16(f("W_in2A").T)
    Win2B_T = b16(f("W_in2B").T)
    in_maps = []
    for c in range(N_CORES):
        m = dict(shared)
        m["xoT"] = b16(xT[:, c * NO:(c + 1) * NO])
        if c < N_CORES // 2:
            m["win_o"] = WinA_T; m["bin_o"] = col("b_inA")
            m["win2_o"] = Win2A_T; m["bin2_o"] = col("b_in2A")
        else:
            m["win_o"] = WinB_T; m["bin_o"] = col("b_inB")
            m["win2_o"] = Win2B_T; m["bin2_o"] = col("b_in2B")
        m.update(tables[c])
        in_maps.append(m)

    if TRACE:
        _install_trace_hook()
    res = run_bass_kernel_spmd(nc, in_maps, list(range(N_CORES)),
                               trace=bool(TRACE))
    LAST_EXEC_NS = res.exec_time_ns
    _LAST_RES = res

    l_full = np.empty((N, H), np.float32)
    g_full = np.empty((N, H), np.float32)
    for c in range(N_CORES):
        r = res.results[c]
        l_full[c * NO:(c + 1) * NO] = r["out_l"].reshape(H, NO).T
        g_full[c * NO:(c + 1) * NO] = r["out_g"].reshape(H, NO).T
    z_A = np.concatenate([l_full[:NA], g_full[:NA]], 1)
    z_B = np.concatenate([l_full[NA:], g_full[NA:]], 1)
    return (z_A, z_B)


# revision 33
# speedup vs baseline: 1.6644x; 1.0028x over previous
"""Trainium2 Bass kernel for nn_GAT_T (2x GATConv + dense self-attention hybrid).

Sharding across 8 NeuronCores: core c owns nodes [1024c, 1024(c+1)).
 - GATConv: dst-node sharded, dense masked-softmax formulation. Per-layer
   h (all nodes) comes from an own-slice matmul + AllGather; the adjacency
   mask is built just-in-time inside the GAT loop by gpsimd local_scatter
   (no DRAM round trip). The LeakyReLU+exp is factored exactly as
   exp(lrelu(ss+sd)) = max(e^ss * e^sd, e^{.2ss} * e^{.2sd}) so the inner
   loop is 3 fused DVE ops (no ScalarE).
 - Dense NxN attention: query-row sharded; K/V own-slice + AllGather during
   GAT1, streamed in 512KB tiles during the attention phase.
 - Collectives (order): ssrc1 (4KB), h1 (1MB/core), k, v during prologue/
   GAT1; ssrc2 (4KB), h2 (1MB/core) during attention.
Heavy matmuls in bf16 with fp32 PSUM accumulation. Softmax computed without
max-subtraction (logits are O(+-6); mathematically identical).
"""

import numpy as np

NA, NB = 4096, 4096
N = NA + NB
IN, H = 256, 512
N_CORES = 8
NO = N // N_CORES      # 1024 nodes per core
KT = N // 128          # 64 src chunks
KP = KT // 2           # 32 src chunk-pairs
NEG_SLOPE = 0.2

TRACE = False
LAST_EXEC_NS = None
_LAST_RES = None
_CACHE = {}


def _install_trace_hook():
    import sys, types
    if "antenv.axon_hooks" in sys.modules:
        return
    try:
        mod = types.ModuleType("antenv.axon_hooks")
        mod._hook = None
        mod.set_axon_ntff_profile_hook = lambda h: setattr(mod, "_hook", h)
        mod.get_axon_ntff_profile_hook = lambda: mod._hook
        sys.modules["antenv.axon_hooks"] = mod
        from trn_agent_boot.trn_boot import _ntff_profile_via_ctypes
        mod.set_axon_ntff_profile_hook(
            _ntff_profile_via_ctypes("/opt/axon/libaxon_pjrt.so"))
    except Exception:
        pass


def _build(W0, W1):
    import concourse.bacc as bacc
    import concourse.mybir as mybir
    import concourse.tile as tile

    f32 = mybir.dt.float32
    bf16 = mybir.dt.bfloat16
    i16 = mybir.dt.int16
    AF = mybir.ActivationFunctionType
    ALU = mybir.AluOpType

    nc = bacc.Bacc("TRN2", target_bir_lowering=False, debug=False,
                   num_devices=N_CORES)

    def inp(name, shape, dt=f32):
        return nc.dram_tensor(name, shape, dt, kind="ExternalInput")

    # weights arrive pre-cast to bf16 from the host so every load is a plain
    # HWDGE DMA (keeps the gpsimd queue free for the barrier + cc triggers)
    xoT = inp("xoT", [IN, NO], bf16)
    xTb = inp("xTb", [IN, N], bf16)
    win_o = inp("win_o", [IN, H], bf16); win2_o = inp("win2_o", [IN, H], bf16)
    bin_o = inp("bin_o", [H, 1]); bin2_o = inp("bin2_o", [H, 1])
    WcA = inp("WcA", [IN, H], bf16); WcB = inp("WcB", [IN, H], bf16)
    SV = inp("SV", [IN, 2], bf16)
    bh1r = inp("bh1r", [2, H]); c0r = inp("c0r", [1, 2])
    WSD1 = inp("WSD1", [H, 2], bf16)
    WSD2 = inp("WSD2", [H, 2], bf16)
    Wg2_T = inp("Wg2_T", [H, H], bf16)
    bg1 = inp("bg1", [H, 1]); bg2 = inp("bg2", [H, 1])
    WqT = inp("WqT", [H, H], bf16); WkT = inp("WkT", [H, H], bf16)
    WvT = inp("WvT", [H, H], bf16)
    bq = inp("bq", [H, 1]); bk = inp("bk", [H, 1]); bv = inp("bv", [H, 1])
    WoT = inp("WoT", [H, H], bf16); bo = inp("bo", [H, 1])
    sc_idx0 = inp("sc_idx0", [128, KP * W0], i16)
    sc_cnt0 = inp("sc_cnt0", [128, KP * W0], bf16)
    sc_idx1 = inp("sc_idx1", [128, KP * W1], i16)
    sc_cnt1 = inp("sc_cnt1", [128, KP * W1], bf16)

    out_l = nc.dram_tensor("out_l", [4, 128, NO], f32, kind="ExternalOutput")
    out_g = nc.dram_tensor("out_g", [4, 128, NO], f32, kind="ExternalOutput")

    RG = [list(range(N_CORES))]
    SCL = 1.0 / float(np.sqrt(H))

    with tile.TileContext(nc) as tc:
        with (
            tc.tile_pool(name="wp", bufs=1) as wp,
            tc.tile_pool(name="apool", bufs=1) as ap,
            tc.tile_pool(name="own", bufs=1) as op_,
            tc.tile_pool(name="hp", bufs=1) as hp,
            tc.tile_pool(name="sp", bufs=3) as sp,
            tc.tile_pool(name="rp", bufs=2) as rp,
            tc.tile_pool(name="lp", bufs=3) as lp,
            tc.tile_pool(name="pp", bufs=1, space="PSUM") as pp,
            tc.tile_pool(name="ppmm", bufs=2, space="PSUM") as ppmm,
            tc.tile_pool(name="dram", bufs=1, space="DRAM") as dp,
        ):
            # ---------- load weights (HWDGE; already bf16) ----------
            def w16(dram, rows, cols, tag):
                t = wp.tile([128, rows // 128, cols], bf16, tag=tag)
                nc.sync.dma_start(
                    t[:], dram[:].rearrange("(a p) c -> p a c", p=128))
                return t

            def bias32(dram, tag):
                t = wp.tile([128, H // 128], f32, tag=tag)
                nc.scalar.dma_start(
                    t[:], dram[:].rearrange("(a p) one -> p (a one)", p=128))
                return t

            # own x first (scalar queue), prologue-critical weights (sync)
            xo = lp.tile([128, 2, NO], bf16, tag="xo", bufs=1)
            nc.scalar.dma_start(
                xo[:], xoT[:].rearrange("(a p) c -> p a c", p=128))
            wo_ = w16(win_o, IN, H, "wo_")
            bof_ = bias32(bin_o, "bof_")
            wsd1 = w16(WSD1, H, 2, "wsd1")
            wcA = w16(WcA, IN, H, "wcA")
            wcB = w16(WcB, IN, H, "wcB")
            svt = w16(SV, IN, 2, "svt")
            bg1f = bias32(bg1, "bg1f")
            c0t = wp.tile([1, 2], f32, tag="c0t")
            nc.scalar.dma_start(c0t[:], c0r[:])
            bh1s = wp.tile([1, 2, H], f32, tag="bh1s")
            nc.scalar.dma_start(
                bh1s[:], bh1r[:].rearrange("(o t) c -> o t c", o=1))
            w2o_ = w16(win2_o, IN, H, "w2o_")
            b2of_ = bias32(bin2_o, "b2of_")
            qT = w16(WqT, H, H, "qT"); bqf = bias32(bq, "bqf")
            kT = w16(WkT, H, H, "kT"); bkf = bias32(bk, "bkf")
            vT = w16(WvT, H, H, "vT")

            ones_r = wp.tile([1, 128], f32, tag="ones_r")
            nc.vector.memset(ones_r[:], 1.0)
            ones_cb = wp.tile([128, 1], bf16, tag="ones_cb")
            nc.vector.memset(ones_cb[:], 1.0)

            sci = [wp.tile([128, KP * W0], i16, tag="sci0", name="sci0"),
                   wp.tile([128, KP * W1], i16, tag="sci1", name="sci1")]
            scc = [wp.tile([128, KP * W0], bf16, tag="scc0", name="scc0"),
                   wp.tile([128, KP * W1], bf16, tag="scc1", name="scc1")]

            # bv as a broadcast row [128, 512] f32
            bvrow = wp.tile([1, H], f32, tag="bvrow")
            nc.scalar.dma_start(bvrow[:], bv[:].rearrange("f o -> o f"))

            # ---------- internal DRAM ----------
            s1_stage = dp.tile([1, N], f32, tag="s1st")
            ko_b = dp.tile([4, 128, NO], bf16, tag="kob")
            kg = dp.tile([N_CORES, 4, 128, NO], bf16, tag="kg",
                         addr_space="Shared")
            vo_b = dp.tile([8, 128, 512], bf16, tag="vob")
            vg = dp.tile([N_CORES, 8, 128, 512], bf16, tag="vg",
                         addr_space="Shared")
            s2o = dp.tile([1, NO], f32, tag="s2o")
            s2g = dp.tile([N_CORES, 1, NO], f32, tag="s2g", addr_space="Shared")
            h2o_b = dp.tile([8, 128, 512], bf16, tag="h2ob")
            h2g = dp.tile([N_CORES, 8, 128, 512], bf16, tag="h2g",
                          addr_space="Shared")

            # ---------- prologue: own input linears ----------
            l0o = op_.tile([128, 4, NO], bf16, tag="l0o")
            g0o = op_.tile([128, 4, NO], bf16, tag="g0o")
            for n2 in range(2):
                for m in range(4):
                    ps = ppmm.tile([128, 512], f32, tag="mm")
                    for k2 in range(2):
                        nc.tensor.matmul(
                            ps[:], lhsT=wo_[:, k2, 128 * m:128 * (m + 1)],
                            rhs=xo[:, k2, 512 * n2:512 * (n2 + 1)],
                            start=(k2 == 0), stop=(k2 == 1))
                    nc.vector.tensor_scalar_add(
                        l0o[:, m, 512 * n2:512 * (n2 + 1)], ps[:], bof_[:, m:m + 1])

            def ssrc_own(wsd, lown, s_dram, queue):
                """own-node ssrc rows -> s_dram [1, NO] f32."""
                for n2 in range(2):
                    psd = pp.tile([1, 512], f32, tag="den")
                    for k2 in range(4):
                        nc.tensor.matmul(
                            psd[:], lhsT=wsd[:, k2, 0:1],
                            rhs=lown[:, k2, 512 * n2:512 * (n2 + 1)],
                            start=(k2 == 0), stop=(k2 == 3))
                    row = rp.tile([1, 512], f32, tag="row")
                    nc.vector.tensor_copy(row[:], psd[:])
                    queue.dma_start(
                        s_dram[:, 512 * n2:512 * (n2 + 1)], row[:])


            # ---------- g0o, q, k, v own + AllGather k/v ----------
            for n2 in range(2):
                for m in range(4):
                    ps = ppmm.tile([128, 512], f32, tag="mm")
                    for k2 in range(2):
                        nc.tensor.matmul(
                            ps[:], lhsT=w2o_[:, k2, 128 * m:128 * (m + 1)],
                            rhs=xo[:, k2, 512 * n2:512 * (n2 + 1)],
                            start=(k2 == 0), stop=(k2 == 1))
                    nc.vector.tensor_scalar_add(
                        g0o[:, m, 512 * n2:512 * (n2 + 1)], ps[:], b2of_[:, m:m + 1])

            q16 = ap.tile([128, 4, NO], bf16, tag="q16")
            for n2 in range(2):
                for m in range(4):
                    ps = ppmm.tile([128, 512], f32, tag="mm")
                    for k2 in range(4):
                        nc.tensor.matmul(
                            ps[:], lhsT=qT[:, k2, 128 * m:128 * (m + 1)],
                            rhs=g0o[:, k2, 512 * n2:512 * (n2 + 1)],
                            start=(k2 == 0), stop=(k2 == 3))
                    nc.vector.tensor_scalar_add(
                        q16[:, m, 512 * n2:512 * (n2 + 1)], ps[:], bqf[:, m:m + 1])

            for n2 in range(2):
                for mp in range(2):
                    st2 = sp.tile([128, 2, 512], bf16, tag="stg2", bufs=2)
                    for mi in range(2):
                        m = 2 * mp + mi
                        ps = ppmm.tile([128, 512], f32, tag="mm")
                        for k2 in range(4):
                            nc.tensor.matmul(
                                ps[:], lhsT=kT[:, k2, 128 * m:128 * (m + 1)],
                                rhs=g0o[:, k2, 512 * n2:512 * (n2 + 1)],
                                start=(k2 == 0), stop=(k2 == 3))
                        nc.vector.tensor_scalar_add(
                            st2[:, mi, :], ps[:], bkf[:, m:m + 1])
                    nc.sync.dma_start(
                        ko_b[2 * mp:2 * mp + 2, :, 512 * n2:512 * (n2 + 1)]
                        .rearrange("a p c -> p a c"), st2[:])

            # bv broadcast [128, 512]
            pb = ppmm.tile([128, H], f32, tag="mm")
            nc.tensor.matmul(pb[:], lhsT=ones_r[:], rhs=bvrow[:], start=True,
                             stop=True)
            bvb = ap.tile([128, H], f32, tag="bvb")
            nc.vector.tensor_copy(bvb[:], pb[:])
            for tp in range(4):
                st2 = sp.tile([128, 2, 512], bf16, tag="stg2", bufs=2)
                for ti in range(2):
                    t = 2 * tp + ti
                    ps = ppmm.tile([128, 512], f32, tag="mm")
                    for k2 in range(4):
                        nc.tensor.matmul(
                            ps[:], lhsT=g0o[:, k2, 128 * t:128 * (t + 1)],
                            rhs=vT[:, k2, :], start=(k2 == 0), stop=(k2 == 3))
                    nc.vector.tensor_add(st2[:, ti, :], ps[:], bvb[:])
                nc.sync.dma_start(
                    vo_b[2 * tp:2 * tp + 2, :, :].rearrange("a p c -> p a c"),
                    st2[:])
            nc.gpsimd.collective_compute(
                "AllGather", mybir.AluOpType.bypass,
                replica_groups=RG, ins=[ko_b.opt()], outs=[kg.opt()])
            nc.gpsimd.collective_compute(
                "AllGather", mybir.AluOpType.bypass,
                replica_groups=RG, ins=[vo_b.opt()], outs=[vg.opt()])

            # layer-2 / attention-epilogue weights, reusing layer-1 slots
            # (issued after the collective triggers so the WAR waits cannot
            # delay them; the donors' last reads are all in the prologue)
            g2T = w16(Wg2_T, H, H, "g1T")
            wsd2 = w16(WSD2, H, 2, "wsd1")
            bg2f = bias32(bg2, "bg2f")
            oT = w16(WoT, H, H, "kT")
            bof2 = bias32(bo, "bof2")

            # ---------- fused stage-0 full: h1 = x @ Wc + bh1 (all nodes) ----
            # Wc = W_in^T Wg1^T is host-fused, so h1 for every node comes
            # straight from x with no l0 staging; runs during the startup
            # barrier + k/v gathers. ssrc1 = x @ SV + c0 likewise.
            hts1 = [hp.tile([128, 8, 512], bf16, tag=f"h{c}", name=f"h1_{c}")
                    for c in range(N_CORES)]
            bh1b = []
            for t in range(2):
                pbh = ppmm.tile([128, H], f32, tag="mm")
                nc.tensor.matmul(pbh[:], lhsT=ones_r[:], rhs=bh1s[0:1, t, :],
                                 start=True, stop=True)
                bb = ap.tile([128, H], bf16, tag=f"bh1b{t}", name=f"bh1b{t}")
                nc.vector.tensor_copy(bb[:], pbh[:])
                bh1b.append(bb)
            for n16 in range(16):
                ta = 0 if n16 < 8 else 1
                wc = wcA if n16 < 8 else wcB
                xq = lp.tile([128, 2, 512], bf16, tag="xq", bufs=2)
                nc.scalar.dma_start(
                    xq[:], xTb[:, 512 * n16:512 * (n16 + 1)]
                    .rearrange("(a p) c -> p a c", p=128))
                for t in range(4):
                    kk = 4 * n16 + t
                    ps = ppmm.tile([128, 512], f32, tag="mm")
                    for k2 in range(2):
                        nc.tensor.matmul(
                            ps[:], lhsT=xq[:, k2, 128 * t:128 * (t + 1)],
                            rhs=wc[:, k2, :], start=(k2 == 0), stop=(k2 == 1))
                    nc.vector.tensor_add(
                        hts1[kk // 8][:, kk % 8, :], ps[:], bh1b[ta][:])
                psd = pp.tile([1, 512], f32, tag="den")
                for k2 in range(2):
                    nc.tensor.matmul(
                        psd[:], lhsT=svt[:, k2, ta:ta + 1], rhs=xq[:, k2, :],
                        start=(k2 == 0), stop=(k2 == 1))
                row = rp.tile([1, 512], f32, tag="row")
                nc.vector.tensor_scalar_add(row[:], psd[:], c0t[0:1, ta:ta + 1])
                nc.sync.dma_start(
                    s1_stage[:, 512 * n16:512 * (n16 + 1)], row[:])
            nc.scalar.dma_start(sci[0][:], sc_idx0[:])
            nc.scalar.dma_start(scc[0][:], sc_cnt0[:])
            nc.scalar.dma_start(sci[1][:], sc_idx1[:])
            nc.scalar.dma_start(scc[1][:], sc_cnt1[:])
            sc1 = ap.tile([128, KT], f32, tag="sc", name="sc1")
            nc.scalar.dma_start(
                sc1[:], s1_stage[0:1, :].rearrange("o (t p) -> p (o t)", p=128))

            # ---------- helpers ----------
            def compute_sdb(wsd, lown, tag):
                """sdst over own nodes, broadcast to [128, NO] f32."""
                sdb = ap.tile([128, NO], bf16, tag=tag)
                for n2 in range(2):
                    psd = pp.tile([1, 512], f32, tag="den")
                    for k2 in range(4):
                        nc.tensor.matmul(
                            psd[:], lhsT=wsd[:, k2, 1:2],
                            rhs=lown[:, k2, 512 * n2:512 * (n2 + 1)],
                            start=(k2 == 0), stop=(k2 == 3))
                    row = rp.tile([1, 512], f32, tag="row")
                    nc.vector.tensor_copy(row[:], psd[:])
                    psb = ppmm.tile([128, 512], f32, tag="mm")
                    nc.tensor.matmul(psb[:], lhsT=ones_r[:], rhs=row[:],
                                     start=True, stop=True)
                    nc.vector.tensor_copy(sdb[:, 512 * n2:512 * (n2 + 1)], psb[:])
                return sdb

            def exp_factors(sdb, sc):
                """F1/F2 [128, NO] bf16 (exp of sdst, exp of .2 sdst) and
                E1/E2 [128, KT] f32 (exp of ssrc)."""
                F1 = ap.tile([128, NO], bf16, tag="F1", name="F1")
                F2 = ap.tile([128, NO], bf16, tag="F2", name="F2")
                nc.scalar.activation(F1[:], sdb[:], AF.Exp)
                nc.scalar.activation(F2[:], sdb[:], AF.Exp, scale=NEG_SLOPE)
                E1 = ap.tile([128, KT], f32, tag="E1", name="E1")
                E2 = ap.tile([128, KT], f32, tag="E2", name="E2")
                nc.scalar.activation(E1[:], sc[:], AF.Exp)
                nc.scalar.activation(E2[:], sc[:], AF.Exp, scale=NEG_SLOPE)
                return F1, F2, E1, E2

            def load_hres(h_g, queue, tag):
                hts = []
                for c in range(N_CORES):
                    ht = hp.tile([128, 8, 512], bf16, tag=f"{tag}{c}")
                    queue.dma_start(
                        ht[:], h_g[c, :, :, :].rearrange("a p c -> p a c"))
                    hts.append(ht)
                return hts

            def gat_loop(hts, F1, F2, E1, E2, sdb, ssc, li, write_out):
                # per chunk-pair: ki=0 attention weights on ScalarE
                # (Prelu+Exp) with the mask multiply on gpsimd; ki=1 on DVE
                # (two fused stt + max). Balances all four engines under PE.
                for j in range(2):
                    Wj = W0 if j == 0 else W1
                    aggs = [pp.tile([128, 512], f32, tag=f"agg{m}",
                                    name=f"agg{li}{m}") for m in range(4)]
                    wsum = rp.tile([128, 512], bf16, tag="wsum")
                    for kp in range(KP):
                        mf = sp.tile([128, 2, 512], bf16, tag="mask", bufs=3)
                        nc.gpsimd.local_scatter(
                            out_ap=mf[:], data_ap=scc[j][:, kp * Wj:(kp + 1) * Wj],
                            idxs_ap=sci[j][:, kp * Wj:(kp + 1) * Wj],
                            channels=128, num_elems=NO, num_idxs=Wj)
                        k0 = 2 * kp
                        et = sp.tile([128, 512], f32, tag="et", bufs=2)
                        nc.scalar.activation(
                            et[:], sdb[:, 512 * j:512 * (j + 1)], AF.Prelu,
                            bias=ssc[:, k0:k0 + 1], scale=1.0, alpha=NEG_SLOPE)
                        pt = sp.tile([128, 512], bf16, tag="pt", bufs=2)
                        nc.scalar.activation(pt[:], et[:], AF.Exp)
                        wt0 = sp.tile([128, 512], bf16, tag="wt0", bufs=3)
                        nc.vector.tensor_mul(wt0[:], pt[:], mf[:, 0, :])
                        ht = hts[k0 // 8][:, k0 % 8, :]
                        for m in range(4):
                            nc.tensor.matmul(
                                aggs[m][:], lhsT=ht[:, 128 * m:128 * (m + 1)],
                                rhs=wt0[:], start=(k0 == 0), stop=False)
                        k1 = 2 * kp + 1
                        u = sp.tile([128, 512], bf16, tag="u", bufs=2)
                        nc.vector.scalar_tensor_tensor(
                            u[:], F1[:, 512 * j:512 * (j + 1)],
                            E1[:, k1:k1 + 1], mf[:, 1, :],
                            op0=ALU.mult, op1=ALU.mult)
                        v2 = sp.tile([128, 512], bf16, tag="v", bufs=2)
                        nc.vector.scalar_tensor_tensor(
                            v2[:], F2[:, 512 * j:512 * (j + 1)],
                            E2[:, k1:k1 + 1], mf[:, 1, :],
                            op0=ALU.mult, op1=ALU.mult)
                        wt1 = sp.tile([128, 512], bf16, tag="wt", bufs=3)
                        nc.vector.tensor_tensor(wt1[:], u[:], v2[:], op=ALU.max)
                        ht = hts[k1 // 8][:, k1 % 8, :]
                        for m in range(4):
                            nc.tensor.matmul(
                                aggs[m][:], lhsT=ht[:, 128 * m:128 * (m + 1)],
                                rhs=wt1[:], start=False, stop=(k1 == KT - 1))
                        if kp == 0:
                            nc.vector.tensor_add(wsum[:], wt0[:], wt1[:])
                        else:
                            wpair = sp.tile([128, 512], bf16, tag="wpair",
                                            bufs=2)
                            nc.vector.tensor_add(wpair[:], wt0[:], wt1[:])
                            nc.vector.tensor_add(wsum[:], wsum[:], wpair[:])
                    den = pp.tile([1, 512], f32, tag="den")
                    nc.tensor.matmul(den[:], lhsT=ones_cb[:], rhs=wsum[:],
                                     start=True, stop=True)
                    denr = rp.tile([1, 512], f32, tag="row")
                    nc.vector.tensor_copy(denr[:], den[:])
                    invp = pp.tile([128, 512], f32, tag="invb")
                    nc.tensor.matmul(invp[:], lhsT=ones_r[:], rhs=denr[:],
                                     start=True, stop=True)
                    invs = rp.tile([128, 512], f32, tag="invs", bufs=1)
                    nc.vector.reciprocal_approx_fast(invs[:], invp[:])
                    for m in range(4):
                        write_out(j, m, aggs[m], invs)

            # ---------- GAT layer 1 ----------
            sdb1 = compute_sdb(wsd1, l0o, "sdb")
            F11, F21, E11, E21 = exp_factors(sdb1, sc1)
            l1own = op_.tile([128, 4, NO], bf16, tag="l1own")

            def write_l1(j, m, agg, invs):
                tmp = sp.tile([128, 512], f32, tag="tmp", bufs=1)
                nc.vector.tensor_mul(tmp[:], agg[:], invs[:])
                nc.vector.tensor_scalar_add(
                    l1own[:, m, 512 * j:512 * (j + 1)], tmp[:], bg1f[:, m:m + 1])

            gat_loop(hts1, F11, F21, E11, E21, sdb1, sc1, 1, write_l1)

            # ---------- ssrc2 + h2 own -> AllGather (fly during attention) ----
            ssrc_own(wsd2, l1own, s2o, nc.gpsimd)
            nc.gpsimd.collective_compute(
                "AllGather", mybir.AluOpType.bypass,
                replica_groups=RG, ins=[s2o.opt()], outs=[s2g.opt()])
            for tp in range(4):
                st2 = sp.tile([128, 2, 512], bf16, tag="stg2", bufs=2)
                for ti in range(2):
                    t = 2 * tp + ti
                    ps = ppmm.tile([128, 512], f32, tag="mm")
                    for k2 in range(4):
                        nc.tensor.matmul(
                            ps[:], lhsT=l1own[:, k2, 128 * t:128 * (t + 1)],
                            rhs=g2T[:, k2, :], start=(k2 == 0), stop=(k2 == 3))
                    nc.vector.tensor_copy(st2[:, ti, :], ps[:])
                nc.gpsimd.dma_start(
                    h2o_b[2 * tp:2 * tp + 2, :, :].rearrange("a p c -> p a c"),
                    st2[:])
            nc.gpsimd.collective_compute(
                "AllGather", mybir.AluOpType.bypass,
                replica_groups=RG, ins=[h2o_b.opt()], outs=[h2g.opt()])

            # ---------- attention ----------
            at16 = ap.tile([128, 4, 512], bf16, tag="at16")
            for qh in range(2):
                avs = [pp.tile([128, 512], f32, tag=f"agg{m}",
                               name=f"av{m}") for m in range(4)]
                esum = rp.tile([128, 512], bf16, tag="wsum")
                for kkp in range(KT // 2):
                    kk0 = 2 * kkp
                    cr = kk0 // 8
                    dl = kk0 % 8
                    ktile = lp.tile([128, 4, 256], bf16, tag="kst", bufs=2)
                    nc.sync.dma_start(
                        ktile[:], kg[cr, :, :, 128 * dl:128 * (dl + 2)]
                        .rearrange("a p c -> p a c"))
                    vtile = lp.tile([128, 2, 512], bf16, tag="vst", bufs=2)
                    nc.scalar.dma_start(
                        vtile[:], vg[cr, dl:dl + 2, :, :]
                        .rearrange("a p c -> p a c"))
                    ess = []
                    for ci in range(2):
                        kk = kk0 + ci
                        pscr = ppmm.tile([128, 512], f32, tag="mm")
                        for k2 in range(4):
                            nc.tensor.matmul(
                                pscr[:],
                                lhsT=ktile[:, k2, 128 * ci:128 * (ci + 1)],
                                rhs=q16[:, k2, 512 * qh:512 * (qh + 1)],
                                start=(k2 == 0), stop=(k2 == 3))
                        es = sp.tile([128, 512], bf16, tag="es", bufs=2)
                        nc.scalar.activation(es[:], pscr[:], AF.Exp, scale=SCL)
                        ess.append(es)
                        for m in range(4):
                            nc.tensor.matmul(
                                avs[m][:],
                                lhsT=vtile[:, ci, 128 * m:128 * (m + 1)],
                                rhs=es[:], start=(kk == 0),
                                stop=(kk == KT - 1))
                    if kkp == 0:
                        nc.vector.tensor_add(esum[:], ess[0][:], ess[1][:])
                    else:
                        epair = sp.tile([128, 512], bf16, tag="wpair", bufs=2)
                        nc.vector.tensor_add(epair[:], ess[0][:], ess[1][:])
                        nc.vector.tensor_add(esum[:], esum[:], epair[:])
                avden = pp.tile([1, 512], f32, tag="den")
                nc.tensor.matmul(avden[:], lhsT=ones_cb[:], rhs=esum[:],
                                 start=True, stop=True)
                denr = rp.tile([1, 512], f32, tag="row")
                nc.vector.tensor_copy(denr[:], avden[:])
                invp = pp.tile([128, 512], f32, tag="invb")
                nc.tensor.matmul(invp[:], lhsT=ones_r[:], rhs=denr[:],
                                 start=True, stop=True)
                invs = rp.tile([128, 512], f32, tag="invs", bufs=1)
                nc.vector.reciprocal_approx_fast(invs[:], invp[:])
                for m in range(4):
                    nc.vector.tensor_mul(at16[:, m, :], avs[m][:], invs[:])
                for m in range(4):
                    ps = ppmm.tile([128, 512], f32, tag="mm")
                    for k2 in range(4):
                        nc.tensor.matmul(
                            ps[:], lhsT=oT[:, k2, 128 * m:128 * (m + 1)],
                            rhs=at16[:, k2, :], start=(k2 == 0), stop=(k2 == 3))
                    stf = sp.tile([128, 512], f32, tag="stgf", bufs=2)
                    nc.vector.tensor_scalar_add(stf[:], ps[:], bof2[:, m:m + 1])
                    nc.scalar.dma_start(
                        out_g[m, :, 512 * qh:512 * (qh + 1)], stf[:])

            # ---------- GAT layer 2 ----------
            sdb2 = compute_sdb(wsd2, l1own, "sdb")
            sc2 = ap.tile([128, KT], f32, tag="sc", name="sc2")
            nc.scalar.dma_start(
                sc2[:], s2g[:].rearrange("c o (a p) -> p (c a o)", p=128))
            F12, F22, E12, E22 = exp_factors(sdb2, sc2)
            hts2 = load_hres(h2g, nc.sync, "h")

            def write_l2(j, m, agg, invs):
                tmp = sp.tile([128, 512], f32, tag="tmp", bufs=1)
                nc.vector.tensor_mul(tmp[:], agg[:], invs[:])
                stf = sp.tile([128, 512], f32, tag="stgf", bufs=2)
                nc.vector.tensor_scalar_add(stf[:], tmp[:], bg2f[:, m:m + 1])
                nc.scalar.dma_start(
                    out_l[m, :, 512 * j:512 * (j + 1)], stf[:])

            gat_loop(hts2, F12, F22, E12, E22, sdb2, sc2, 2, write_l2)

    nc.finalize()
    return nc


def _prep_tables(src, dst):
    """Per-core, per-dst-half scatter tables for JIT mask construction.

    For dst half j of core c (512 dst nodes), edges are bucketed by
    (src chunk-pair kp = src//256, src partition p = src%128); the scatter
    writes count values at column (src//128 % 2)*512 + (dst - base) of a
    [128, 1024] tile covering src chunks 2kp, 2kp+1."""
    import ml_dtypes
    per = {0: [], 1: []}
    Wmax = [0, 0]
    for c in range(N_CORES):
        for j in range(2):
            lo = c * NO + 512 * j
            sel = (dst >= lo) & (dst < lo + 512)
            s = src[sel].astype(np.int64)
            dcol = (dst[sel] - lo).astype(np.int64)
            key = s * 512 + dcol
            uniq, counts = np.unique(key, return_counts=True)
            s_u = uniq // 512
            col = (uniq % 512) + 512 * ((s_u // 128) % 2)
            kp = s_u // 256
            p = s_u % 128
            bucket = kp * 128 + p
            order = np.argsort(bucket, kind="stable")
            bucket = bucket[order]
            col = col[order]
            counts = counts[order]
            bstart = np.r_[0, np.flatnonzero(np.diff(bucket)) + 1]
            sizes = np.diff(np.r_[bstart, bucket.size])
            slot = np.arange(bucket.size) - np.repeat(bstart, sizes)
            Wmax[j] = max(Wmax[j], int(sizes.max()) if sizes.size else 0)
            per[j].append((bucket, col, counts, slot))
    Ws = [max(2, (w + 1) // 2 * 2) for w in Wmax]
    tables = []
    for c in range(N_CORES):
        t = {}
        for j in range(2):
            W = Ws[j]
            bucket, col, counts, slot = per[j][c]
            sc_idx = np.full((128, KP * W), -1, np.int16)
            sc_cnt = np.zeros((128, KP * W), ml_dtypes.bfloat16)
            kp = bucket // 128
            p = bucket % 128
            flat = kp * W + slot
            sc_idx[p, flat] = col.astype(np.int16)
            sc_cnt[p, flat] = counts.astype(np.float32)
            t[f"sc_idx{j}"] = sc_idx
            t[f"sc_cnt{j}"] = sc_cnt
        tables.append(t)
    return Ws[0], Ws[1], tables


def kernel(**inputs):
    global LAST_EXEC_NS, _LAST_RES
    from concourse.bass_utils import run_bass_kernel_spmd

    f = lambda name: np.ascontiguousarray(np.asarray(inputs[name], np.float32))
    x_A, x_B = f("x_A"), f("x_B")
    eAB = np.asarray(inputs["edge_AB"]).astype(np.int64)
    eBA = np.asarray(inputs["edge_BA"]).astype(np.int64)

    src = np.concatenate([eAB[0], eBA[0] + NA, np.arange(N, dtype=np.int64)])
    dst = np.concatenate([eAB[1] + NA, eBA[1], np.arange(N, dtype=np.int64)])
    W0, W1, tables = _prep_tables(src, dst)

    if (W0, W1) not in _CACHE:
        _CACHE[(W0, W1)] = _build(W0, W1)
    nc = _CACHE[(W0, W1)]

    import ml_dtypes
    b16 = lambda a: np.ascontiguousarray(np.asarray(a, ml_dtypes.bfloat16))
    xT = np.ascontiguousarray(np.concatenate([x_A, x_B], 0).T)
    col = lambda name: f(name).reshape(-1, 1)
    WqkvT = f("Wqkv").T  # [H, 3H]
    # host-fused stage-0 weights (f64 for a single rounding step)
    Wg1d = np.float64(f("Wg1"))
    WcA_ = np.float64(f("W_inA")).T @ Wg1d.T
    WcB_ = np.float64(f("W_inB")).T @ Wg1d.T
    bh1A = Wg1d @ np.float64(f("b_inA"))
    bh1B = Wg1d @ np.float64(f("b_inB"))
    asrc1 = np.float64(f("a_src1"))
    shared = {
        "xTb": b16(xT),
        "WcA": b16(WcA_), "WcB": b16(WcB_),
        "SV": b16(np.stack([WcA_ @ asrc1, WcB_ @ asrc1], 1)),
        "bh1r": np.ascontiguousarray(
            np.stack([bh1A, bh1B], 0).astype(np.float32)),
        "c0r": np.array([[bh1A @ asrc1, bh1B @ asrc1]], np.float32),
        "WSD1": b16(np.stack([Wg1d.T @ asrc1,
                              Wg1d.T @ np.float64(f("a_dst1"))], 1)),
        "WSD2": b16(np.stack([np.float64(f("Wg2")).T @ np.float64(f("a_src2")),
                              np.float64(f("Wg2")).T @ np.float64(f("a_dst2"))],
                             1)),
        "bg1": col("bg1"),
        "Wg2_T": b16(f("Wg2").T),
        "bg2": col("bg2"),
        "WqT": b16(WqkvT[:, 0:H]),
        "WkT": b16(WqkvT[:, H:2 * H]),
        "WvT": b16(WqkvT[:, 2 * H:3 * H]),
        "bq": col("bqkv")[0:H], "bk": col("bqkv")[H:2 * H],
        "bv": col("bqkv")[2 * H:3 * H],
        "WoT": b16(f("Wo").T), "bo": col("bo"),
    }
    WinA_T = b16(f("W_inA").T)
    WinB_T = b16(f("W_inB").T)
    Win2A_T = b16(f("W_in2A").T)
    Win2B_T = b16(f("W_in2B").T)
    in_maps = []
    for c in range(N_CORES):
        m = dict(shared)
        m["xoT"] = b16(xT[:, c * NO:(c + 1) * NO])
        if c < N_CORES // 2:
            m["win_o"] = WinA_T; m["bin_o"] = col("b_inA")
            m["win2_o"] = Win2A_T; m["bin2_o"] = col("b_in2A")
        else:
            m["win_o"] = WinB_T; m["bin_o"] = col("b_inB")
            m["win2_o"] = Win2B_T; m["bin2_o"] = col("b_in2B")
        m.update(tables[c])
        in_maps.append(m)

    if TRACE:
        _install_trace_hook()
    res = run_bass_kernel_spmd(nc, in_maps, list(range(N_CORES)),
                               trace=bool(TRACE))
    LAST_EXEC_NS = res.exec_time_ns
    _LAST_RES = res

    l_full = np.empty((N, H), np.float32)
    g_full = np.empty((N, H), np.float32)
    for c in range(N_CORES):
        r = res.results[c]
        l_full[c * NO:(c + 1) * NO] = r["out_l"].reshape(H, NO).T
        g_full[c * NO:(c + 1) * NO] = r["out_g"].reshape(H, NO).T
    z_A = np.concatenate([l_full[:NA], g_full[:NA]], 1)
    z_B = np.concatenate([l_full[NA:], g_full[NA:]], 1)
    return (z_A, z_B)
